# revision 9
# baseline (speedup 1.0000x reference)
"""NMR FID simulation kernel for 8 Trainium2 NeuronCores (Bass/Tile).

Math: the reference's sequential scan  rho_{t+1} = P rho_t P^dag,
FID[t] = tr(Op rho_t)  is restructured as a parallel-in-time blocked
Heisenberg evaluation:

    FID[t0 + 16q + r] = tr(Op_r rho_{t0+16q}),   Op_r = K_r^dag Op K_r,
    K_r = P^r  (r < 16),  rho advanced in blocks of 16 via K16 = P^16.

Each of the 8 cores computes 512 consecutive samples (4 rho chains x 8
block-snapshots x 16 in-block samples), then a partial DFT of its own
512 samples (3-stage factorized; fftshift and the per-core global
twiddle folded into constants). Host gathers: concat FID chunks, sum
the 8 partial spectra.

Structure exploited (all verified against the reference numerics):
  - H0 is real symmetric; built on device from h_mat via a
    Walsh-Hadamard factorization of the XOR-structured flip-flop
    couplings (3 matmuls + a few vector ops).
  - P = expm(-i H0 dt) = cos(M) - i sin(M), M = H0*dt, |M| ~ 0.02 ->
    short Taylor series in W = M^2 is exact to fp32.
  - P and all its powers are complex symmetric -> matmul stationary
    operands (lhsT) need no transposes anywhere.
  - rho is Hermitian; stored as (Re rho, -Im rho): rho^T = conj(rho),
    so the stored tiles ARE the lhsT of rho (Re sym, Im antisym).
  - Op = sum_i I+_i is real; rho_0 = U90y IHz U90y^dag = +IHx exactly.
"""

import functools
import numpy as np

N_SPINS = 7
DIM = 128
NS = 4096
DT = 1e-5
B0 = 80.0
T2 = 1.0
SW = 1000.0
N_CORES = 8

# per-stage reduced-precision (float32r) switches for the PE-heavy stages
F32R_ADVANCE = True
F32R_OPR = True
F32R_FFT = True


# ----------------------------------------------------------------------
# host constants (input-independent, same role as reference's _IX etc.)
# ----------------------------------------------------------------------
@functools.lru_cache(maxsize=1)
def _consts():
    f32 = np.float32

    def spin_ops():
        sx = np.array([[0, 1], [1, 0]], dtype=np.complex128) / 2
        sy = np.array([[0, -1j], [1j, 0]], dtype=np.complex128) / 2
        sz = np.array([[1, 0], [0, -1]], dtype=np.complex128) / 2

        def embed(s):
            ops = []
            for i in range(N_SPINS):
                left = np.eye(2**i) if i > 0 else np.array(1.0)
                right = (
                    np.eye(2 ** (N_SPINS - i - 1))
                    if N_SPINS - i - 1 > 0
                    else np.array(1.0)
                )
                ops.append(np.kron(np.kron(left, s), right))
            return np.stack(ops)

        return embed(sx), embed(sy), embed(sz)

    IX, IY, IZ = spin_ops()
    OP = (IX.sum(0) + 1j * IY.sum(0)).real  # total raising operator, real
    RHO0 = IX.sum(0).real                    # U90y IHz U90y^dag == +IHx

    pair = (
        np.einsum('iab,jbc->ijac', IX, IX)
        + np.einsum('iab,jbc->ijac', IY, IY)
        + np.einsum('iab,jbc->ijac', IZ, IZ)
    ).real

    def bitmask(i):
        return 1 << (N_SPINS - 1 - i)

    pairs = [(i, j) for i in range(7) for j in range(i + 1, 7)]
    masks = [bitmask(i) ^ bitmask(j) for (i, j) in pairs]

    BV = np.zeros((DIM, DIM))
    for (i, j), m in zip(pairs, masks):
        FF = pair[i, j].copy()
        np.fill_diagonal(FF, 0.0)
        BV += 2 * np.pi * FF
    DB = np.zeros((28, DIM))
    for i in range(7):
        DB[i] = 2 * np.pi * B0 * np.diag(IZ[i].real)
    for k, (i, j) in enumerate(pairs):
        DB[7 + k] = 2 * np.pi * np.diag(pair[i, j])
    SEL49 = np.zeros((49, 28))
    for i in range(7):
        SEL49[i * 7 + i, i] = 1.0
    for k, (i, j) in enumerate(pairs):
        SEL49[i * 7 + j, 7 + k] = 1.0
    SELM = np.zeros((28, DIM))
    for k, m in enumerate(masks):
        SELM[7 + k, m] = 1.0
    a = np.arange(DIM)
    HAD = np.where(
        np.array([[bin(x & y).count('1') for y in a] for x in a]) % 2 == 0,
        1.0,
        -1.0,
    )

    # extraction row-selectors: T1 = even rows of Px, T2 = odd rows
    SELP = np.zeros((64, 64))
    for s in range(32):
        SELP[2 * s, s] = 1.0
        SELP[2 * s + 1, 32 + s] = 1.0

    s_ = np.arange(32)
    k3 = np.arange(512)
    F512 = np.exp(-2j * np.pi * np.outer(s_, k3) / 512)
    r_ = np.arange(16)
    k4 = np.arange(16)
    F16s = np.exp(-2j * np.pi * np.outer(r_, (k4 + 8) % 16) / 16)  # fftshift folded

    c = {
        'ident': np.eye(DIM),
        'opt': OP.T.copy(),
        'had': HAD,
        'hadn': HAD / DIM,
        'bvdt': BV * DT,   # DT folded: device builds M = H0*DT directly
        'dbdt': DB * DT,
        'sel49': SEL49,
        'selm': SELM,
        'rho0': RHO0,
        'selp': SELP,
        'far': F512.real.copy(),
        'fai': F512.imag.copy(),
        'fcr': F16s.real.copy(),
        'fci': F16s.imag.copy(),
        'fain': -F512.imag,
        'fcin': -F16s.imag,
    }
    return {k: np.ascontiguousarray(v, dtype=f32) for k, v in c.items()}


def _core_inputs(h_mat, core):
    """Per-core config: sharding metadata, apod window, folded twiddles."""
    f32 = np.float32
    t0 = 512 * core
    bits = np.zeros((DIM, 9), f32)
    for b in range(3):
        v = (core >> b) & 1
        bits[:, b] = v
        bits[:, 3 + b] = 1 - v
        bits[:, 6 + b] = -v
    s_idx = np.arange(32)[:, None]
    r_idx = np.arange(16)[None, :]
    apod = np.exp((-DT / T2) * (t0 + 16 * s_idx + r_idx)).astype(f32)
    r_ = np.arange(16)
    k3 = np.arange(512)
    TWc = np.exp(-2j * np.pi * np.outer(r_, k3) / 8192) * np.exp(
        -2j * np.pi * core * k3[None, :] / 16
    )
    return {
        'h49': np.ascontiguousarray(h_mat, dtype=f32).reshape(49, 1),
        'bits': bits,
        'apod': apod,
        'twr': TWc.real.astype(f32).copy(),
        'twi': TWc.imag.astype(f32).copy(),
    }


CONST_SHAPES = {
    'ident': (DIM, DIM), 'opt': (DIM, DIM), 'had': (DIM, DIM),
    'hadn': (DIM, DIM), 'bvdt': (DIM, DIM), 'dbdt': (28, DIM),
    'sel49': (49, 28), 'selm': (28, DIM), 'rho0': (DIM, DIM),
    'selp': (64, 64), 'far': (32, 512), 'fai': (32, 512),
    'fcr': (16, 16), 'fci': (16, 16),
    'fain': (32, 512), 'fcin': (16, 16),
    'h49': (49, 1), 'bits': (DIM, 9), 'apod': (32, 16),
    'twr': (16, 512), 'twi': (16, 512),
}


# ----------------------------------------------------------------------
# bass program
# ----------------------------------------------------------------------
@functools.lru_cache(maxsize=1)
def _build_nc():
    import concourse.mybir as mybir
    import concourse.tile as tile
    from concourse import bacc

    F32 = mybir.dt.float32
    F32R = mybir.dt.float32r
    ALU = mybir.AluOpType

    def adv(ap):   # advance-stage matmul operand view
        if not F32R_ADVANCE:
            return ap
        return ap if ap.dtype == F32R else ap.bitcast(F32R)

    def opr(ap):
        if not F32R_OPR:
            return ap
        return ap if ap.dtype == F32R else ap.bitcast(F32R)

    def fft(ap):
        if not F32R_FFT:
            return ap
        return ap if ap.dtype == F32R else ap.bitcast(F32R)

    nc = bacc.Bacc(None, target_bir_lowering=False)

    F32R_CONSTS = {'opt', 'far', 'fai', 'fain', 'fcr', 'fci', 'fcin'}
    dram = {
        name: nc.dram_tensor(name, list(shape),
                             F32R if name in F32R_CONSTS else F32,
                             kind="ExternalInput")
        for name, shape in CONST_SHAPES.items()
    }
    out_fidr = nc.dram_tensor("fidr", [32, 16], F32, kind="ExternalOutput")
    out_fidi = nc.dram_tensor("fidi", [32, 16], F32, kind="ExternalOutput")
    out_spr = nc.dram_tensor("spr", [16, 512], F32, kind="ExternalOutput")
    out_spi = nc.dram_tensor("spi", [16, 512], F32, kind="ExternalOutput")

    with tile.TileContext(nc) as tc:
        with (
            tc.tile_pool(name="persist", bufs=1) as pp,
            tc.tile_pool(name="psA", bufs=4, space="PSUM") as ppA,   # 4 banks
            tc.tile_pool(name="psB", bufs=2, space="PSUM") as ppB,   # 2 banks
            tc.tile_pool(name="psX", bufs=1, space="PSUM") as ppX,   # 1 bank
        ):
            sb = {}
            for name, shape in CONST_SHAPES.items():
                dt_ = F32R if name in F32R_CONSTS else F32
                sb[name] = pp.tile(list(shape), dt_, tag=name, name=name)
                nc.sync.dma_start(sb[name][:], dram[name][:])

            def ptile(tag, shape):
                t = pp.tile(list(shape), F32, tag=tag, name=tag)
                sb[tag] = t
                return t

            KLr = ptile("KLr", [DIM, 16 * DIM])    # K_r real, r = 0..15
            KLi = ptile("KLi", [DIM, 16 * DIM])
            nKLi = ptile("nKLi", [DIM, 16 * DIM])  # -KLi (rounded)
            KLrR = ptile("KLrR", [DIM, 16 * DIM])   # rounded K_r copies
            KLiR = ptile("KLiR", [DIM, 16 * DIM])
            MLr = ptile("MLr", [DIM, 16 * DIM])    # Op @ K_r
            MLi = ptile("MLi", [DIM, 16 * DIM])
            OST = ptile("OST", [DIM, 2 * 16 * DIM])   # Op_r stack (pp, r, b)
            STK = ptile("STK", [DIM, 32 * 2 * DIM])   # rho snapshots (s, p, b)
            # big powers, blocks p=0..7: K16,K32,K64,K128,K256,K512,A2,A4
            KB = ptile("KB", [DIM, 8 * 256])    # ( Kr |  Ki)
            KBn = ptile("KBn", [DIM, 8 * 256])  # (-Kr | -Ki)
            KBm = ptile("KBm", [DIM, 8 * 256])  # ( Kr | -Ki)  step-1 rhs
            KBs = ptile("KBs", [DIM, 8 * 256])  # ( Ki |  Kr)  step-1 rhs
            KBq = ptile("KBq", [DIM, 8 * 128])  # rounded Kr, step-3 lhsT
            PIDX = {16: 0, 32: 1, 64: 2, 128: 3, 256: 4, 512: 5, 1024: 6, 2048: 7}
            Zr = ptile("Zr", [DIM, 512])
            Zi = ptile("Zi", [DIM, 512])
            RWr = ptile("RWr", [DIM, 512])   # working rho chains (contiguous)
            RWi = ptile("RWi", [DIM, 512])   # stores -Im(rho)

            def kb_r(p): return KB[:, 256 * PIDX[p]: 256 * PIDX[p] + 128]
            def kb_i(p): return KB[:, 256 * PIDX[p] + 128: 256 * PIDX[p] + 256]
            def kbn_r(p): return KBn[:, 256 * PIDX[p]: 256 * PIDX[p] + 128]
            def kbn_i(p): return KBn[:, 256 * PIDX[p] + 128: 256 * PIDX[p] + 256]
            def kbm_ri(p): return KBm[:, 256 * PIDX[p]: 256 * PIDX[p] + 256]
            def kbq_r(p): return KBq[:, 128 * PIDX[p]: 128 * PIDX[p] + 128]
            def kbs_ri(p): return KBs[:, 256 * PIDX[p]: 256 * PIDX[p] + 256]
            def kl_r(r): return KLr[:, 128 * r: 128 * r + 128]
            def kl_i(r): return KLi[:, 128 * r: 128 * r + 128]
            def klr_r(r): return KLrR[:, 128 * r: 128 * r + 128]
            def klr_i(r): return KLiR[:, 128 * r: 128 * r + 128]
            def nkl_i(r): return nKLi[:, 128 * r: 128 * r + 128]

            _evac_flip = [0]

            def evac(dst, src):
                # PSUM -> SBUF, alternating engines for balance
                if _evac_flip[0] % 2 == 0:
                    nc.vector.tensor_copy(dst, src)
                else:
                    nc.scalar.copy(dst, src)
                _evac_flip[0] += 1

            def evac_r(dst, src):
                # PSUM -> SBUF with f32r rounding on write
                d = dst.bitcast(F32R) if F32R_ADVANCE else dst
                if _evac_flip[0] % 2 == 0:
                    nc.vector.tensor_copy(d, src)
                else:
                    nc.scalar.copy(d, src)
                _evac_flip[0] += 1

            def scratch_neg(src):
                t = pp.tile([DIM, DIM], F32, tag="negscr", name="negscr", bufs=4)
                nc.vector.tensor_scalar_mul(t[:], src, -1.0)
                return t

            # ============ stage 1: h_mat -> M = H0*DT (Walsh build) ========
            with nc.named_scope("h0"):
                psT = ppX.tile([DIM, 512], F32, tag="px", name="psT")
                nc.tensor.matmul(psT[0:28, 0:1], sb['sel49'][:], sb['h49'][:],
                                 start=True, stop=True)
                coef = ptile("coef", [28, 1])
                nc.vector.tensor_copy(coef[:], psT[0:28, 0:1])
                nc.tensor.matmul(psT[:, 4:5], sb['selm'][:], coef[:],
                                 start=True, stop=True)
                cmask = ptile("cmask", [DIM, 1])
                nc.vector.tensor_copy(cmask[:], psT[:, 4:5])
                nc.tensor.matmul(psT[:, 8:9], sb['hadn'][:], cmask[:],
                                 start=True, stop=True)
                chat = ptile("chat", [DIM, 1])
                nc.vector.tensor_copy(chat[:], psT[:, 8:9])
                nc.tensor.matmul(psT[:, 12:13], sb['dbdt'][:], coef[:],
                                 start=True, stop=True)
                dvec = ptile("dvec", [DIM, 1])
                nc.vector.tensor_copy(dvec[:], psT[:, 12:13])
                Dc = ptile("Dc", [DIM, DIM])
                nc.vector.tensor_scalar_mul(Dc[:], sb['had'][:], chat[:])
                nc.tensor.matmul(psT[:, 128:256], sb['had'][:], Dc[:],
                                 start=True, stop=True)
                CBsb = ptile("CBsb", [DIM, DIM])
                nc.vector.tensor_copy(CBsb[:], psT[:, 128:256])
                M1 = ptile("M1", [DIM, DIM])
                nc.vector.tensor_mul(M1[:], CBsb[:], sb['bvdt'][:])
                M2 = ptile("M2", [DIM, DIM])
                nc.vector.tensor_scalar_mul(M2[:], sb['ident'][:], dvec[:])
                Mt = ptile("Mt", [DIM, DIM])
                nc.vector.tensor_add(Mt[:], M1[:], M2[:])

            # ============ stage 2: Taylor P = cos(M) - i sin(M) ============
            with nc.named_scope("taylor"):
                ps = ppA.tile([DIM, 512], F32, tag="mm", name="psW")
                nc.tensor.matmul(ps[:, 0:128], Mt[:], Mt[:], start=True, stop=True)
                W = ptile("W", [DIM, DIM])
                evac(W[:], ps[:, 0:128])
                ps = ppA.tile([DIM, 512], F32, tag="mm", name="psW2")
                nc.tensor.matmul(ps[:, 0:128], W[:], W[:], start=True, stop=True)
                W2 = ptile("W2", [DIM, DIM])
                evac(W2[:], ps[:, 0:128])
                ps = ppA.tile([DIM, 512], F32, tag="mm", name="psT3")
                nc.tensor.matmul(ps[:, 0:128], W[:], W2[:], start=True, stop=True)
                T3 = ptile("T3", [DIM, DIM])
                evac(T3[:], ps[:, 0:128])
                # K1 real = I - W/2 + W2/24 - T3/720
                t1 = ptile("tay1", [DIM, DIM])
                nc.vector.scalar_tensor_tensor(
                    t1[:], W[:], -0.5, sb['ident'][:], ALU.mult, ALU.add)
                t2 = ptile("tay2", [DIM, DIM])
                nc.vector.scalar_tensor_tensor(
                    t2[:], W2[:], 1.0 / 24, t1[:], ALU.mult, ALU.add)
                nc.vector.scalar_tensor_tensor(
                    KLr[:, 128:256], T3[:], -1.0 / 720, t2[:], ALU.mult, ALU.add)
                # negE = W/6 - I - W2/120 + T3/5040 ; K1 imag = M @ negE
                t3 = ptile("tay3", [DIM, DIM])
                nc.vector.scalar_tensor_tensor(
                    t3[:], W[:], 1.0 / 6, sb['ident'][:], ALU.mult, ALU.subtract)
                t4 = ptile("tay4", [DIM, DIM])
                nc.vector.scalar_tensor_tensor(
                    t4[:], W2[:], -1.0 / 120, t3[:], ALU.mult, ALU.add)
                negE = ptile("negE", [DIM, DIM])
                nc.vector.scalar_tensor_tensor(
                    negE[:], T3[:], 1.0 / 5040, t4[:], ALU.mult, ALU.add)
                ps = ppA.tile([DIM, 512], F32, tag="mm", name="psKi")
                nc.tensor.matmul(ps[:, 0:128], Mt[:], negE[:], start=True, stop=True)
                evac(KLi[:, 128:256], ps[:, 0:128])
                nc.vector.tensor_copy(KLr[:, 0:128], sb['ident'][:])
                nc.vector.memset(KLi[:, 0:128], 0.0)

            # ============ stage 3: ladder ============
            def rnd(ap):
                return ap.bitcast(F32R) if F32R_ADVANCE else ap

            def arrange_kb(p, mixed):
                """Rounded f32r arrangements of block p for the advance stage."""
                nc.vector.tensor_scalar_mul(rnd(kbn_r(p)), kb_r(p), -1.0)
                nc.vector.tensor_scalar_mul(rnd(kbn_i(p)), kb_i(p), -1.0)
                if mixed:
                    nc.scalar.copy(rnd(kbq_r(p)), kb_r(p))
                    ri = kbm_ri(p)
                    nc.vector.tensor_copy(rnd(ri[:, 0:128]), kb_r(p))
                    nc.vector.tensor_scalar_mul(rnd(ri[:, 128:256]),
                                                kb_i(p), -1.0)
                    ri = kbs_ri(p)
                    nc.scalar.copy(rnd(ri[:, 0:128]), kb_i(p))
                    nc.scalar.copy(rnd(ri[:, 128:256]), kb_r(p))

            with nc.named_scope("ladder"):
                for n in [1, 2, 4, 8]:
                    lo, hi = n + 1, min(2 * n, 16)
                    w = (hi - lo + 1) * 128
                    nki = scratch_neg(kl_i(n))
                    for base in range(0, w, 512):
                        cw = min(512, w - base)
                        boff = 128 + base
                        psR = ppA.tile([DIM, 512], F32, tag="mm", name="psLr")
                        psI = ppA.tile([DIM, 512], F32, tag="mm", name="psLi")
                        nc.tensor.matmul(psR[:, 0:cw], kl_r(n),
                                         KLr[:, boff:boff + cw],
                                         start=True, stop=False)
                        nc.tensor.matmul(psR[:, 0:cw], nki[:],
                                         KLi[:, boff:boff + cw],
                                         start=False, stop=True)
                        nc.tensor.matmul(psI[:, 0:cw], kl_i(n),
                                         KLr[:, boff:boff + cw],
                                         start=True, stop=False)
                        nc.tensor.matmul(psI[:, 0:cw], kl_r(n),
                                         KLi[:, boff:boff + cw],
                                         start=False, stop=True)
                        doff = 128 * lo + base
                        dw = min(cw, 2048 - doff) if doff < 2048 else 0
                        if dw > 0:
                            evac(KLr[:, doff:doff + dw], psR[:, 0:dw])
                            evac(KLi[:, doff:doff + dw], psI[:, 0:dw])
                        if dw < cw:      # tail target is K16 -> KB block 0
                            evac(kb_r(16), psR[:, dw:cw])
                            evac(kb_i(16), psI[:, dw:cw])
                arrange_kb(16, mixed=True)
                for p in [32, 64, 128, 256, 512, 1024, 2048]:
                    h = p // 2
                    nkh = scratch_neg(kb_i(h))
                    ps = ppA.tile([DIM, 512], F32, tag="mm", name="psSq")
                    nc.tensor.matmul(ps[:, 0:128], kb_r(h), kb_r(h),
                                     start=True, stop=False)
                    nc.tensor.matmul(ps[:, 0:128], nkh[:], kb_i(h),
                                     start=False, stop=True)
                    nc.tensor.matmul(ps[:, 128:256], kb_i(h), kb_r(h),
                                     start=True, stop=False)
                    nc.tensor.matmul(ps[:, 128:256], kb_r(h), kb_i(h),
                                     start=False, stop=True)
                    evac(kb_r(p), ps[:, 0:128])
                    evac(kb_i(p), ps[:, 128:256])
                    if p in (128, 256):
                        arrange_kb(p, mixed=True)
                if F32R_OPR:
                    nc.scalar.mul(nKLi[:].bitcast(F32R), KLi[:], -1.0)
                    nc.vector.tensor_copy(KLrR[:].bitcast(F32R), KLr[:])
                    nc.scalar.copy(KLiR[:].bitcast(F32R), KLi[:])
                else:
                    nc.scalar.mul(nKLi[:], KLi[:], -1.0)
                    nc.vector.tensor_copy(KLrR[:], KLr[:])
                    nc.scalar.copy(KLiR[:], KLi[:])

            # ============ stage 4: per-core Q & rho start ============
            with nc.named_scope("start"):
                Ssel = ptile("Ssel", [DIM, 6 * 128])    # (Sr|Si) x 3
                nSsel = ptile("nSsel", [DIM, 3 * 128])  # -Si x 3
                for b, p in enumerate([512, 1024, 2048]):
                    tb = ptile("selt%d" % b, [DIM, DIM])
                    nc.vector.tensor_scalar_mul(
                        tb[:], sb['ident'][:], sb['bits'][:, 3 + b:4 + b])
                    nc.vector.scalar_tensor_tensor(
                        Ssel[:, 256 * b:256 * b + 128], kb_r(p),
                        sb['bits'][:, b:b + 1], tb[:], ALU.mult, ALU.add)
                    nc.vector.tensor_scalar_mul(
                        Ssel[:, 256 * b + 128:256 * b + 256], kb_i(p),
                        sb['bits'][:, b:b + 1])
                    nc.vector.tensor_scalar_mul(
                        nSsel[:, 128 * b:128 * b + 128], kb_i(p),
                        sb['bits'][:, 6 + b:7 + b])

                def cmul_sym(dstR, dstI, Ar, Ai, nAi, Br, Bi,
                             neg_out_i=False, nAr=None, round_out=False):
                    if round_out and F32R_ADVANCE:
                        dstR = dstR.bitcast(F32R)
                        dstI = dstI.bitcast(F32R)
                    ps = ppA.tile([DIM, 512], F32, tag="mm", name="psCm")
                    nc.tensor.matmul(ps[:, 0:128], Ar, Br, start=True, stop=False)
                    nc.tensor.matmul(ps[:, 0:128], nAi, Bi, start=False, stop=True)
                    if neg_out_i:
                        nc.tensor.matmul(ps[:, 128:256], nAi, Br,
                                         start=True, stop=False)
                        nc.tensor.matmul(ps[:, 128:256], nAr, Bi,
                                         start=False, stop=True)
                    else:
                        nc.tensor.matmul(ps[:, 128:256], Ai, Br,
                                         start=True, stop=False)
                        nc.tensor.matmul(ps[:, 128:256], Ar, Bi,
                                         start=False, stop=True)
                    evac(dstR, ps[:, 0:128])
                    evac(dstI, ps[:, 128:256])

                C12r = ptile("C12r", [DIM, DIM])
                C12i = ptile("C12i", [DIM, DIM])
                cmul_sym(C12r[:], C12i[:],
                         Ssel[:, 0:128], Ssel[:, 128:256], nSsel[:, 0:128],
                         Ssel[:, 256:384], Ssel[:, 384:512])
                nC12i = scratch_neg(C12i[:])
                Qr = ptile("Qr", [DIM, DIM])
                Qi = ptile("Qi", [DIM, DIM])
                cmul_sym(Qr[:], Qi[:], C12r[:], C12i[:], nC12i[:],
                         Ssel[:, 512:640], Ssel[:, 640:768])
                nQr = scratch_neg(Qr[:])
                nQi = scratch_neg(Qi[:])
                # Y = rho0 @ conj(Q): Yr = rho0@Qr ; Yi = -rho0@Qi
                psY = ppA.tile([DIM, 512], F32, tag="mm", name="psY")
                nc.tensor.matmul(psY[:, 0:128], sb['rho0'][:], Qr[:],
                                 start=True, stop=True)
                nc.tensor.matmul(psY[:, 128:256], sb['rho0'][:], Qi[:],
                                 start=True, stop=True)
                Yr = ptile("Yr", [DIM, DIM])
                Yi = ptile("Yi", [DIM, DIM])
                nc.vector.tensor_copy(Yr[:], psY[:, 0:128])
                nc.vector.tensor_scalar_mul(Yi[:], psY[:, 128:256], -1.0)
                # rho = Q @ Y -> working chain j=0 as (Rr, -Ri)
                cmul_sym(RWr[:, 0:128], RWi[:, 0:128],
                         Qr[:], Qi[:], nQi[:], Yr[:], Yi[:],
                         neg_out_i=True, nAr=nQr[:], round_out=True)

            # ============ stage 5: spawns + q-loop ============
            # STK layout (b-major): col = 64*b + 16*j + 2*q + p
            def stk_view(js, q, p):
                # AP [128, len(js), 128(b)] at snapshot cols for chains js
                v = STK[:].rearrange("p (b j s x) -> p b j s x", b=DIM, j=4, s=8)
                j0 = js[0]
                jc = len(js)
                jstep = (js[1] - j0) if jc > 1 else 1
                vv = v[:, :, j0:j0 + jc * jstep:jstep if jc > 1 else 1, q, p]
                return vv

            def archive(js, q):
                # DMA working chains js (at step q) into STK snapshot cols
                v = STK[:].rearrange("p (b j s x) -> p b j s x", b=DIM, j=4, s=8)
                if True:
                    for j in js:
                        nc.sync.dma_start(
                            v[:, :, j, q, 0],
                            RWr[:, 128 * j:128 * j + 128])
                        nc.sync.dma_start(
                            v[:, :, j, q, 1],
                            RWi[:, 128 * j:128 * j + 128])

            def advance_round(src_js, dst_js, q_dst, p, inplace_q=None):
                width = len(src_js)
                w = width * 128
                psZ = None
                for idx, j in enumerate(src_js):
                    lR = RWr[:, 128 * j:128 * j + 128]
                    lN = RWi[:, 128 * j:128 * j + 128]
                    if idx % 2 == 0:
                        psZ = ppA.tile([DIM, 512], F32, tag="mm", name="psZ")
                    sl = psZ[:, (idx % 2) * 256: (idx % 2) * 256 + 256]
                    nc.tensor.matmul(sl, adv(lR), adv(kbm_ri(p)),
                                     start=True, stop=False)
                    nc.tensor.matmul(sl, adv(lN), adv(kbs_ri(p)),
                                     start=False, stop=True)
                    evac_r(Zr[:, idx * 128:(idx + 1) * 128], sl[:, 0:128])
                    evac_r(Zi[:, idx * 128:(idx + 1) * 128], sl[:, 128:256])
                psR = ppB.tile([DIM, 512], F32, tag="mm2", name="psR")
                psN = ppB.tile([DIM, 512], F32, tag="mm2", name="psN")
                nc.tensor.matmul(psR[:, 0:w], adv(kbq_r(p)), adv(Zr[:, 0:w]),
                                 start=True, stop=False)
                nc.tensor.matmul(psR[:, 0:w], adv(kbn_i(p)), adv(Zi[:, 0:w]),
                                 start=False, stop=True)
                nc.tensor.matmul(psN[:, 0:w], adv(kbn_r(p)), adv(Zi[:, 0:w]),
                                 start=True, stop=False)
                nc.tensor.matmul(psN[:, 0:w], adv(kbn_i(p)), adv(Zr[:, 0:w]),
                                 start=False, stop=True)
                for idx, j in enumerate(dst_js):
                    evac_r(RWr[:, 128 * j:128 * j + 128],
                           psR[:, 128 * idx:128 * idx + 128])
                    evac_r(RWi[:, 128 * j:128 * j + 128],
                           psN[:, 128 * idx:128 * idx + 128])

            with nc.named_scope("evolve"):
                archive([0], 0)                      # s = 0
                advance_round([0], [2], None, 256)   # j2 = j0 + 256
                archive([2], 0)                      # s = 16
                advance_round([0, 2], [1, 3], None, 128)
                archive([1], 0)                      # s = 8
                archive([3], 0)                      # s = 24
                for q in range(1, 8):
                    advance_round([0, 1, 2, 3], [0, 1, 2, 3], None, 16)
                    archive([0, 1, 2, 3], q)

            # ============ stage 6: Op_r stack ============
            with nc.named_scope("opr"):
                for b4 in range(4):
                    lo = 512 * b4
                    psm = ppA.tile([DIM, 512], F32, tag="mm", name="psm")
                    psn = ppA.tile([DIM, 512], F32, tag="mm", name="psn")
                    nc.tensor.matmul(psm[:, 0:512], opr(sb['opt'][:]),
                                     opr(KLrR[:, lo:lo + 512]),
                                     start=True, stop=True)
                    nc.tensor.matmul(psn[:, 0:512], opr(sb['opt'][:]),
                                     opr(KLiR[:, lo:lo + 512]),
                                     start=True, stop=True)
                    if F32R_OPR:
                        evac(MLr[:, lo:lo + 512].bitcast(F32R), psm[:, 0:512])
                        evac(MLi[:, lo:lo + 512].bitcast(F32R), psn[:, 0:512])
                    else:
                        evac(MLr[:, lo:lo + 512], psm[:, 0:512])
                        evac(MLi[:, lo:lo + 512], psn[:, 0:512])
                for r in range(16):
                    c0 = 128 * r
                    pso = ppA.tile([DIM, 512], F32, tag="mm", name="pso")
                    nc.tensor.matmul(pso[:, 0:128], opr(klr_r(r)),
                                     opr(MLr[:, c0:c0 + 128]),
                                     start=True, stop=False)
                    nc.tensor.matmul(pso[:, 0:128], opr(klr_i(r)),
                                     opr(MLi[:, c0:c0 + 128]),
                                     start=False, stop=True)
                    nc.tensor.matmul(pso[:, 128:256], opr(klr_r(r)),
                                     opr(MLi[:, c0:c0 + 128]),
                                     start=True, stop=False)
                    nc.tensor.matmul(pso[:, 128:256], opr(nkl_i(r)),
                                     opr(MLr[:, c0:c0 + 128]),
                                     start=False, stop=True)
                    ostv = OST[:].rearrange("p (b x r) -> p b x r", b=DIM, x=2)
                    evac(ostv[:, :, 0, r], pso[:, 0:128])
                    evac(ostv[:, :, 1, r], pso[:, 128:256])

            # ============ stage 7: extraction ============
            with nc.named_scope("extract"):
                psPx = ppX.tile([64, 32], F32, tag="px", name="psPx")
                for b in range(DIM):
                    nc.tensor.matmul(psPx[:], STK[:, 64 * b:64 * b + 64],
                                     OST[:, 32 * b:32 * b + 32],
                                     start=(b == 0), stop=(b == DIM - 1))
                Pxsb = ptile("Pxsb", [64, 32])
                nc.vector.tensor_copy(Pxsb[:], psPx[:])
                pst12 = ppB.tile([32, 64], F32, tag="mm2", name="pst12")
                nc.tensor.matmul(pst12[:, 0:32], sb['selp'][:, 0:32], Pxsb[:],
                                 start=True, stop=True)
                nc.tensor.matmul(pst12[:, 32:64], sb['selp'][:, 32:64], Pxsb[:],
                                 start=True, stop=True)
                T12 = ptile("T12", [32, 64])
                nc.vector.tensor_copy(T12[:], pst12[:])
                fra = ptile("fra", [32, 16])
                fia = ptile("fia", [32, 16])
                fre = ptile("fre", [32, 16])
                fim = ptile("fim", [32, 16])
                nc.vector.tensor_sub(fra[:], T12[:, 0:16], T12[:, 48:64])
                nc.vector.tensor_add(fia[:], T12[:, 16:32], T12[:, 32:48])
                if F32R_FFT:
                    nc.vector.tensor_mul(fre[:].bitcast(F32R), fra[:],
                                         sb['apod'][:])
                    nc.vector.tensor_mul(fim[:].bitcast(F32R), fia[:],
                                         sb['apod'][:])
                else:
                    nc.vector.tensor_mul(fre[:], fra[:], sb['apod'][:])
                    nc.vector.tensor_mul(fim[:], fia[:], sb['apod'][:])
                nc.sync.dma_start(out_fidr[:], fre[:])
                nc.sync.dma_start(out_fidi[:], fim[:])

            # ============ stage 8: partial spectrum (3-stage DFT) ============
            with nc.named_scope("fft"):
                psYa = ppB.tile([16, 512], F32, tag="mm2", name="psYa")
                nc.tensor.matmul(psYa[:], fft(fre[:]), fft(sb['far'][:]),
                                 start=True, stop=False)
                nc.tensor.matmul(psYa[:], fft(fim[:]), fft(sb['fain'][:]),
                                 start=False, stop=True)
                Yfr = ptile("Yfr", [16, 512])
                nc.vector.tensor_copy(Yfr[:], psYa[:])
                psYb = ppB.tile([16, 512], F32, tag="mm2", name="psYb")
                nc.tensor.matmul(psYb[:], fft(fre[:]), fft(sb['fai'][:]),
                                 start=True, stop=False)
                nc.tensor.matmul(psYb[:], fft(fim[:]), fft(sb['far'][:]),
                                 start=False, stop=True)
                Yfi = ptile("Yfi", [16, 512])
                nc.vector.tensor_copy(Yfi[:], psYb[:])
                # Z = Y o TW (elementwise complex, SBUF only)
                za = ptile("za", [16, 512])
                zb = ptile("zb", [16, 512])
                Zfr = ptile("Zfr", [16, 512])
                Zfi = ptile("Zfi", [16, 512])
                nc.vector.tensor_mul(za[:], Yfr[:], sb['twr'][:])
                nc.vector.tensor_mul(zb[:], Yfi[:], sb['twi'][:])
                nc.vector.tensor_sub(
                    Zfr[:].bitcast(F32R) if F32R_FFT else Zfr[:],
                    za[:], zb[:])
                nc.vector.tensor_mul(za[:], Yfr[:], sb['twi'][:])
                nc.vector.tensor_mul(zb[:], Yfi[:], sb['twr'][:])
                nc.vector.tensor_add(
                    Zfi[:].bitcast(F32R) if F32R_FFT else Zfi[:],
                    za[:], zb[:])
                psSa = ppB.tile([16, 512], F32, tag="mm2", name="psSa")
                nc.tensor.matmul(psSa[:], fft(sb['fcr'][:]), fft(Zfr[:]),
                                 start=True, stop=False)
                nc.tensor.matmul(psSa[:], fft(sb['fcin'][:]), fft(Zfi[:]),
                                 start=False, stop=True)
                spR = ptile("spR", [16, 512])
                nc.vector.tensor_copy(spR[:], psSa[:])
                psSb = ppB.tile([16, 512], F32, tag="mm2", name="psSb")
                nc.tensor.matmul(psSb[:], fft(sb['fci'][:]), fft(Zfr[:]),
                                 start=True, stop=False)
                nc.tensor.matmul(psSb[:], fft(sb['fcr'][:]), fft(Zfi[:]),
                                 start=False, stop=True)
                spI = ptile("spI", [16, 512])
                nc.vector.tensor_copy(spI[:], psSb[:])
                nc.sync.dma_start(out_spr[:], spR[:])
                nc.sync.dma_start(out_spi[:], spI[:])

    nc.compile()
    return nc


# ----------------------------------------------------------------------
# entry point
# ----------------------------------------------------------------------
def kernel(h_mat, n_samp):
    from concourse.bass_utils import run_bass_kernel_spmd

    n_samp = int(n_samp)
    assert n_samp == NS, f"kernel compiled for n_samp={NS}, got {n_samp}"
    h_mat = np.asarray(h_mat, dtype=np.float32)
    assert h_mat.shape == (7, 7)

    nc = _build_nc()
    consts = _consts()
    in_maps = []
    for c in range(N_CORES):
        m = dict(consts)
        m.update(_core_inputs(h_mat, c))
        in_maps.append(m)
    res = run_bass_kernel_spmd(nc, in_maps, core_ids=list(range(N_CORES)))

    # gather / unshard
    fid = np.zeros(NS, dtype=np.complex64)
    spec = np.zeros(2 * NS, dtype=np.complex64)
    for c in range(N_CORES):
        r = res.results[c]
        fid[512 * c: 512 * (c + 1)] = (
            r["fidr"].reshape(-1) + 1j * r["fidi"].reshape(-1))
        spec += r["spr"].reshape(-1) + 1j * r["spi"].reshape(-1)

    aq = NS / SW
    time_series = np.linspace(0.0, aq, NS, dtype=np.float32)
    freq_series = np.linspace(-SW / 2, SW / 2, 2 * NS, dtype=np.float32)
    return fid, time_series, spec, freq_series


# revision 10
# speedup vs baseline: 4.2836x; 4.2836x over previous
"""NMR FID simulation kernel for 8 Trainium2 NeuronCores (Bass/Tile).

Math: the reference's sequential scan  rho_{t+1} = P rho_t P^dag,
FID[t] = tr(Op rho_t)  is restructured as a parallel-in-time blocked
Heisenberg evaluation:

    FID[t0 + 16q + r] = tr(Op_r rho_{t0+16q}),   Op_r = K_r^dag Op K_r,
    K_r = P^r  (r < 16),  rho advanced in blocks of 16 via K16 = P^16.

Each of the 8 cores computes 512 consecutive samples (4 rho chains x 8
block-snapshots x 16 in-block samples), then a partial DFT of its own
512 samples (3-stage factorized; fftshift and the per-core global
twiddle folded into constants). Host gathers: concat FID chunks, sum
the 8 partial spectra.

Structure exploited (all verified against the reference numerics):
  - H0 is real symmetric; built on device from h_mat via a
    Walsh-Hadamard factorization of the XOR-structured flip-flop
    couplings (3 matmuls + a few vector ops).
  - P = expm(-i H0 dt) = cos(M) - i sin(M), M = H0*dt, |M| ~ 0.02 ->
    short Taylor series in W = M^2 is exact to fp32.
  - P and all its powers are complex symmetric -> matmul stationary
    operands (lhsT) need no transposes anywhere.
  - rho is Hermitian; stored as (Re rho, -Im rho): rho^T = conj(rho),
    so the stored tiles ARE the lhsT of rho (Re sym, Im antisym).
  - Op = sum_i I+_i is real; rho_0 = U90y IHz U90y^dag = +IHx exactly.
"""

import functools
import numpy as np

N_SPINS = 7
DIM = 128
NS = 4096
DT = 1e-5
B0 = 80.0
T2 = 1.0
SW = 1000.0
N_CORES = 8

# per-stage reduced-precision (float32r) switches for the PE-heavy stages
F32R_ADVANCE = True
F32R_OPR = True
F32R_FFT = True


# ----------------------------------------------------------------------
# host constants (input-independent, same role as reference's _IX etc.)
# ----------------------------------------------------------------------
@functools.lru_cache(maxsize=1)
def _consts():
    f32 = np.float32

    def spin_ops():
        sx = np.array([[0, 1], [1, 0]], dtype=np.complex128) / 2
        sy = np.array([[0, -1j], [1j, 0]], dtype=np.complex128) / 2
        sz = np.array([[1, 0], [0, -1]], dtype=np.complex128) / 2

        def embed(s):
            ops = []
            for i in range(N_SPINS):
                left = np.eye(2**i) if i > 0 else np.array(1.0)
                right = (
                    np.eye(2 ** (N_SPINS - i - 1))
                    if N_SPINS - i - 1 > 0
                    else np.array(1.0)
                )
                ops.append(np.kron(np.kron(left, s), right))
            return np.stack(ops)

        return embed(sx), embed(sy), embed(sz)

    IX, IY, IZ = spin_ops()
    OP = (IX.sum(0) + 1j * IY.sum(0)).real  # total raising operator, real
    RHO0 = IX.sum(0).real                    # U90y IHz U90y^dag == +IHx

    pair = (
        np.einsum('iab,jbc->ijac', IX, IX)
        + np.einsum('iab,jbc->ijac', IY, IY)
        + np.einsum('iab,jbc->ijac', IZ, IZ)
    ).real

    def bitmask(i):
        return 1 << (N_SPINS - 1 - i)

    pairs = [(i, j) for i in range(7) for j in range(i + 1, 7)]
    masks = [bitmask(i) ^ bitmask(j) for (i, j) in pairs]

    BV = np.zeros((DIM, DIM))
    for (i, j), m in zip(pairs, masks):
        FF = pair[i, j].copy()
        np.fill_diagonal(FF, 0.0)
        BV += 2 * np.pi * FF
    DB = np.zeros((28, DIM))
    for i in range(7):
        DB[i] = 2 * np.pi * B0 * np.diag(IZ[i].real)
    for k, (i, j) in enumerate(pairs):
        DB[7 + k] = 2 * np.pi * np.diag(pair[i, j])
    SEL49 = np.zeros((49, 28))
    for i in range(7):
        SEL49[i * 7 + i, i] = 1.0
    for k, (i, j) in enumerate(pairs):
        SEL49[i * 7 + j, 7 + k] = 1.0
    SELM = np.zeros((28, DIM))
    for k, m in enumerate(masks):
        SELM[7 + k, m] = 1.0
    a = np.arange(DIM)
    HAD = np.where(
        np.array([[bin(x & y).count('1') for y in a] for x in a]) % 2 == 0,
        1.0,
        -1.0,
    )

    # extraction row-selectors: T1 = even rows of Px, T2 = odd rows
    SELP = np.zeros((64, 64))
    for s in range(32):
        SELP[2 * s, s] = 1.0
        SELP[2 * s + 1, 32 + s] = 1.0

    s_ = np.arange(32)
    k3 = np.arange(512)
    F512 = np.exp(-2j * np.pi * np.outer(s_, k3) / 512)
    r_ = np.arange(16)
    k4 = np.arange(16)
    F16s = np.exp(-2j * np.pi * np.outer(r_, (k4 + 8) % 16) / 16)  # fftshift folded

    c = {
        'ident': np.eye(DIM),
        'opt': OP.T.copy(),
        'had': HAD,
        'hadn': HAD / DIM,
        'bvdt': BV * DT,   # DT folded: device builds M = H0*DT directly
        'dbdt': DB * DT,
        'sel49': SEL49,
        'selm': SELM,
        'rho0': RHO0,
        'selp': SELP,
        'far': F512.real.copy(),
        'fai': F512.imag.copy(),
        'fcr': F16s.real.copy(),
        'fci': F16s.imag.copy(),
        'fain': -F512.imag,
        'fcin': -F16s.imag,
    }
    return {k: np.ascontiguousarray(v, dtype=f32) for k, v in c.items()}


def _core_inputs(h_mat, core):
    """Per-core config: sharding metadata, apod window, folded twiddles."""
    f32 = np.float32
    t0 = 512 * core
    bits = np.zeros((DIM, 9), f32)
    for b in range(3):
        v = (core >> b) & 1
        bits[:, b] = v
        bits[:, 3 + b] = 1 - v
        bits[:, 6 + b] = -v
    s_idx = np.arange(32)[:, None]
    r_idx = np.arange(16)[None, :]
    apod = np.exp((-DT / T2) * (t0 + 16 * s_idx + r_idx)).astype(f32)
    r_ = np.arange(16)
    k3 = np.arange(512)
    TWc = np.exp(-2j * np.pi * np.outer(r_, k3) / 8192) * np.exp(
        -2j * np.pi * core * k3[None, :] / 16
    )
    return {
        'h49': np.ascontiguousarray(h_mat, dtype=f32).reshape(49, 1),
        'bits': bits,
        'apod': apod,
        'twr': TWc.real.astype(f32).copy(),
        'twi': TWc.imag.astype(f32).copy(),
    }


CONST_SHAPES = {
    'ident': (DIM, DIM), 'opt': (DIM, DIM), 'had': (DIM, DIM),
    'hadn': (DIM, DIM), 'bvdt': (DIM, DIM), 'dbdt': (28, DIM),
    'sel49': (49, 28), 'selm': (28, DIM), 'rho0': (DIM, DIM),
    'selp': (64, 64), 'far': (32, 512), 'fai': (32, 512),
    'fcr': (16, 16), 'fci': (16, 16),
    'fain': (32, 512), 'fcin': (16, 16),
    'h49': (49, 1), 'bits': (DIM, 9), 'apod': (32, 16),
    'twr': (16, 512), 'twi': (16, 512),
}


# ----------------------------------------------------------------------
# bass program
# ----------------------------------------------------------------------
@functools.lru_cache(maxsize=1)
def _build_nc():
    import concourse.mybir as mybir
    import concourse.tile as tile
    from concourse import bacc

    F32 = mybir.dt.float32
    F32R = mybir.dt.float32r
    ALU = mybir.AluOpType

    def adv(ap):   # advance-stage matmul operand view
        if not F32R_ADVANCE:
            return ap
        return ap if ap.dtype == F32R else ap.bitcast(F32R)

    def opr(ap):
        if not F32R_OPR:
            return ap
        return ap if ap.dtype == F32R else ap.bitcast(F32R)

    def fft(ap):
        if not F32R_FFT:
            return ap
        return ap if ap.dtype == F32R else ap.bitcast(F32R)

    nc = bacc.Bacc(None, target_bir_lowering=False)

    F32R_CONSTS = {'opt', 'far', 'fai', 'fain', 'fcr', 'fci', 'fcin'}
    dram = {
        name: nc.dram_tensor(name, list(shape),
                             F32R if name in F32R_CONSTS else F32,
                             kind="ExternalInput")
        for name, shape in CONST_SHAPES.items()
    }
    out_fidr = nc.dram_tensor("fidr", [32, 16], F32, kind="ExternalOutput")
    out_fidi = nc.dram_tensor("fidi", [32, 16], F32, kind="ExternalOutput")
    out_spr = nc.dram_tensor("spr", [16, 512], F32, kind="ExternalOutput")
    out_spi = nc.dram_tensor("spi", [16, 512], F32, kind="ExternalOutput")

    with tile.TileContext(nc) as tc:
        with (
            tc.tile_pool(name="persist", bufs=1) as pp,
            tc.tile_pool(name="psA", bufs=4, space="PSUM") as ppA,   # 4 banks
            tc.tile_pool(name="psB", bufs=2, space="PSUM") as ppB,   # 2 banks
            tc.tile_pool(name="psX", bufs=1, space="PSUM") as ppX,   # 1 bank
        ):
            sb = {}
            for name, shape in CONST_SHAPES.items():
                dt_ = F32R if name in F32R_CONSTS else F32
                sb[name] = pp.tile(list(shape), dt_, tag=name, name=name)
                nc.sync.dma_start(sb[name][:], dram[name][:])

            def ptile(tag, shape):
                t = pp.tile(list(shape), F32, tag=tag, name=tag)
                sb[tag] = t
                return t

            KLr = ptile("KLr", [DIM, 16 * DIM])    # K_r real, r = 0..15
            KLi = ptile("KLi", [DIM, 16 * DIM])
            nKLi = ptile("nKLi", [DIM, 16 * DIM])  # -KLi (rounded)
            KLrR = ptile("KLrR", [DIM, 16 * DIM])   # rounded K_r copies
            KLiR = ptile("KLiR", [DIM, 16 * DIM])
            MLr = ptile("MLr", [DIM, 16 * DIM])    # Op @ K_r
            MLi = ptile("MLi", [DIM, 16 * DIM])
            OST = ptile("OST", [DIM, 2 * 16 * DIM])   # Op_r stack (pp, r, b)
            STK = ptile("STK", [DIM, 32 * 2 * DIM])   # rho snapshots (s, p, b)
            # big powers, blocks p=0..7: K16,K32,K64,K128,K256,K512,A2,A4
            KB = ptile("KB", [DIM, 8 * 256])    # ( Kr |  Ki)
            KBn = ptile("KBn", [DIM, 8 * 256])  # (-Kr | -Ki)
            KBm = ptile("KBm", [DIM, 8 * 256])  # ( Kr | -Ki)  step-1 rhs
            KBs = ptile("KBs", [DIM, 8 * 256])  # ( Ki |  Kr)  step-1 rhs
            KBq = ptile("KBq", [DIM, 8 * 128])  # rounded Kr, step-3 lhsT
            PIDX = {16: 0, 32: 1, 64: 2, 128: 3, 256: 4, 512: 5, 1024: 6, 2048: 7}
            Zr = ptile("Zr", [DIM, 512])
            Zi = ptile("Zi", [DIM, 512])

            def kb_r(p): return KB[:, 256 * PIDX[p]: 256 * PIDX[p] + 128]
            def kb_i(p): return KB[:, 256 * PIDX[p] + 128: 256 * PIDX[p] + 256]
            def kbn_r(p): return KBn[:, 256 * PIDX[p]: 256 * PIDX[p] + 128]
            def kbn_i(p): return KBn[:, 256 * PIDX[p] + 128: 256 * PIDX[p] + 256]
            def kbm_ri(p): return KBm[:, 256 * PIDX[p]: 256 * PIDX[p] + 256]
            def kbq_r(p): return KBq[:, 128 * PIDX[p]: 128 * PIDX[p] + 128]
            def kbs_ri(p): return KBs[:, 256 * PIDX[p]: 256 * PIDX[p] + 256]
            def kl_r(r): return KLr[:, 128 * r: 128 * r + 128]
            def kl_i(r): return KLi[:, 128 * r: 128 * r + 128]
            def klr_r(r): return KLrR[:, 128 * r: 128 * r + 128]
            def klr_i(r): return KLiR[:, 128 * r: 128 * r + 128]
            def nkl_i(r): return nKLi[:, 128 * r: 128 * r + 128]

            _evac_flip = [0]

            def evac(dst, src):
                # PSUM -> SBUF, alternating engines for balance
                if _evac_flip[0] % 2 == 0:
                    nc.vector.tensor_copy(dst, src)
                else:
                    nc.scalar.copy(dst, src)
                _evac_flip[0] += 1

            def evac_r(dst, src):
                # PSUM -> SBUF with f32r rounding on write
                d = dst.bitcast(F32R) if F32R_ADVANCE else dst
                if _evac_flip[0] % 2 == 0:
                    nc.vector.tensor_copy(d, src)
                else:
                    nc.scalar.copy(d, src)
                _evac_flip[0] += 1

            def scratch_neg(src):
                t = pp.tile([DIM, DIM], F32, tag="negscr", name="negscr", bufs=4)
                nc.vector.tensor_scalar_mul(t[:], src, -1.0)
                return t

            # ============ stage 1: h_mat -> M = H0*DT (Walsh build) ========
            with nc.named_scope("h0"):
                psT = ppX.tile([DIM, 512], F32, tag="px", name="psT")
                nc.tensor.matmul(psT[0:28, 0:1], sb['sel49'][:], sb['h49'][:],
                                 start=True, stop=True)
                coef = ptile("coef", [28, 1])
                nc.vector.tensor_copy(coef[:], psT[0:28, 0:1])
                nc.tensor.matmul(psT[:, 4:5], sb['selm'][:], coef[:],
                                 start=True, stop=True)
                cmask = ptile("cmask", [DIM, 1])
                nc.vector.tensor_copy(cmask[:], psT[:, 4:5])
                nc.tensor.matmul(psT[:, 8:9], sb['hadn'][:], cmask[:],
                                 start=True, stop=True)
                chat = ptile("chat", [DIM, 1])
                nc.vector.tensor_copy(chat[:], psT[:, 8:9])
                nc.tensor.matmul(psT[:, 12:13], sb['dbdt'][:], coef[:],
                                 start=True, stop=True)
                dvec = ptile("dvec", [DIM, 1])
                nc.vector.tensor_copy(dvec[:], psT[:, 12:13])
                Dc = ptile("Dc", [DIM, DIM])
                nc.vector.tensor_scalar_mul(Dc[:], sb['had'][:], chat[:])
                nc.tensor.matmul(psT[:, 128:256], sb['had'][:], Dc[:],
                                 start=True, stop=True)
                CBsb = ptile("CBsb", [DIM, DIM])
                nc.vector.tensor_copy(CBsb[:], psT[:, 128:256])
                M1 = ptile("M1", [DIM, DIM])
                nc.vector.tensor_mul(M1[:], CBsb[:], sb['bvdt'][:])
                M2 = ptile("M2", [DIM, DIM])
                nc.vector.tensor_scalar_mul(M2[:], sb['ident'][:], dvec[:])
                Mt = ptile("Mt", [DIM, DIM])
                nc.vector.tensor_add(Mt[:], M1[:], M2[:])

            # ============ stage 2: Taylor P = cos(M) - i sin(M) ============
            with nc.named_scope("taylor"):
                ps = ppA.tile([DIM, 512], F32, tag="mm", name="psW")
                nc.tensor.matmul(ps[:, 0:128], Mt[:], Mt[:], start=True, stop=True)
                W = ptile("W", [DIM, DIM])
                evac(W[:], ps[:, 0:128])
                ps = ppA.tile([DIM, 512], F32, tag="mm", name="psW2")
                nc.tensor.matmul(ps[:, 0:128], W[:], W[:], start=True, stop=True)
                W2 = ptile("W2", [DIM, DIM])
                evac(W2[:], ps[:, 0:128])
                ps = ppA.tile([DIM, 512], F32, tag="mm", name="psT3")
                nc.tensor.matmul(ps[:, 0:128], W[:], W2[:], start=True, stop=True)
                T3 = ptile("T3", [DIM, DIM])
                evac(T3[:], ps[:, 0:128])
                # K1 real = I - W/2 + W2/24 - T3/720
                t1 = ptile("tay1", [DIM, DIM])
                nc.vector.scalar_tensor_tensor(
                    t1[:], W[:], -0.5, sb['ident'][:], ALU.mult, ALU.add)
                t2 = ptile("tay2", [DIM, DIM])
                nc.vector.scalar_tensor_tensor(
                    t2[:], W2[:], 1.0 / 24, t1[:], ALU.mult, ALU.add)
                nc.vector.scalar_tensor_tensor(
                    KLr[:, 128:256], T3[:], -1.0 / 720, t2[:], ALU.mult, ALU.add)
                # negE = W/6 - I - W2/120 + T3/5040 ; K1 imag = M @ negE
                t3 = ptile("tay3", [DIM, DIM])
                nc.vector.scalar_tensor_tensor(
                    t3[:], W[:], 1.0 / 6, sb['ident'][:], ALU.mult, ALU.subtract)
                t4 = ptile("tay4", [DIM, DIM])
                nc.vector.scalar_tensor_tensor(
                    t4[:], W2[:], -1.0 / 120, t3[:], ALU.mult, ALU.add)
                negE = ptile("negE", [DIM, DIM])
                nc.vector.scalar_tensor_tensor(
                    negE[:], T3[:], 1.0 / 5040, t4[:], ALU.mult, ALU.add)
                ps = ppA.tile([DIM, 512], F32, tag="mm", name="psKi")
                nc.tensor.matmul(ps[:, 0:128], Mt[:], negE[:], start=True, stop=True)
                evac(KLi[:, 128:256], ps[:, 0:128])
                nc.vector.tensor_copy(KLr[:, 0:128], sb['ident'][:])
                nc.vector.memset(KLi[:, 0:128], 0.0)

            # ============ stage 3: ladder ============
            def rnd(ap):
                return ap.bitcast(F32R) if F32R_ADVANCE else ap

            def arrange_kb(p, mixed):
                """Rounded f32r arrangements of block p for the advance stage."""
                nc.vector.tensor_scalar_mul(rnd(kbn_r(p)), kb_r(p), -1.0)
                nc.vector.tensor_scalar_mul(rnd(kbn_i(p)), kb_i(p), -1.0)
                if mixed:
                    nc.scalar.copy(rnd(kbq_r(p)), kb_r(p))
                    ri = kbm_ri(p)
                    nc.vector.tensor_copy(rnd(ri[:, 0:128]), kb_r(p))
                    nc.vector.tensor_scalar_mul(rnd(ri[:, 128:256]),
                                                kb_i(p), -1.0)
                    ri = kbs_ri(p)
                    nc.scalar.copy(rnd(ri[:, 0:128]), kb_i(p))
                    nc.scalar.copy(rnd(ri[:, 128:256]), kb_r(p))

            with nc.named_scope("ladder"):
                for n in [1, 2, 4, 8]:
                    lo, hi = n + 1, min(2 * n, 16)
                    w = (hi - lo + 1) * 128
                    nki = scratch_neg(kl_i(n))
                    for base in range(0, w, 512):
                        cw = min(512, w - base)
                        boff = 128 + base
                        psR = ppA.tile([DIM, 512], F32, tag="mm", name="psLr")
                        psI = ppA.tile([DIM, 512], F32, tag="mm", name="psLi")
                        nc.tensor.matmul(psR[:, 0:cw], kl_r(n),
                                         KLr[:, boff:boff + cw],
                                         start=True, stop=False)
                        nc.tensor.matmul(psR[:, 0:cw], nki[:],
                                         KLi[:, boff:boff + cw],
                                         start=False, stop=True)
                        nc.tensor.matmul(psI[:, 0:cw], kl_i(n),
                                         KLr[:, boff:boff + cw],
                                         start=True, stop=False)
                        nc.tensor.matmul(psI[:, 0:cw], kl_r(n),
                                         KLi[:, boff:boff + cw],
                                         start=False, stop=True)
                        doff = 128 * lo + base
                        dw = min(cw, 2048 - doff) if doff < 2048 else 0
                        if dw > 0:
                            evac(KLr[:, doff:doff + dw], psR[:, 0:dw])
                            evac(KLi[:, doff:doff + dw], psI[:, 0:dw])
                        if dw < cw:      # tail target is K16 -> KB block 0
                            evac(kb_r(16), psR[:, dw:cw])
                            evac(kb_i(16), psI[:, dw:cw])
                arrange_kb(16, mixed=True)
                for p in [32, 64, 128, 256, 512, 1024, 2048]:
                    h = p // 2
                    nkh = scratch_neg(kb_i(h))
                    ps = ppA.tile([DIM, 512], F32, tag="mm", name="psSq")
                    nc.tensor.matmul(ps[:, 0:128], kb_r(h), kb_r(h),
                                     start=True, stop=False)
                    nc.tensor.matmul(ps[:, 0:128], nkh[:], kb_i(h),
                                     start=False, stop=True)
                    nc.tensor.matmul(ps[:, 128:256], kb_i(h), kb_r(h),
                                     start=True, stop=False)
                    nc.tensor.matmul(ps[:, 128:256], kb_r(h), kb_i(h),
                                     start=False, stop=True)
                    evac(kb_r(p), ps[:, 0:128])
                    evac(kb_i(p), ps[:, 128:256])
                    if p in (128, 256):
                        arrange_kb(p, mixed=True)
                if F32R_OPR:
                    nc.scalar.mul(nKLi[:].bitcast(F32R), KLi[:], -1.0)
                    nc.vector.tensor_copy(KLrR[:].bitcast(F32R), KLr[:])
                    nc.scalar.copy(KLiR[:].bitcast(F32R), KLi[:])
                else:
                    nc.scalar.mul(nKLi[:], KLi[:], -1.0)
                    nc.vector.tensor_copy(KLrR[:], KLr[:])
                    nc.scalar.copy(KLiR[:], KLi[:])

            # ============ stage 4: per-core Q & rho start ============
            with nc.named_scope("start"):
                Ssel = ptile("Ssel", [DIM, 6 * 128])    # (Sr|Si) x 3
                nSsel = ptile("nSsel", [DIM, 3 * 128])  # -Si x 3
                for b, p in enumerate([512, 1024, 2048]):
                    tb = ptile("selt%d" % b, [DIM, DIM])
                    nc.vector.tensor_scalar_mul(
                        tb[:], sb['ident'][:], sb['bits'][:, 3 + b:4 + b])
                    nc.vector.scalar_tensor_tensor(
                        Ssel[:, 256 * b:256 * b + 128], kb_r(p),
                        sb['bits'][:, b:b + 1], tb[:], ALU.mult, ALU.add)
                    nc.vector.tensor_scalar_mul(
                        Ssel[:, 256 * b + 128:256 * b + 256], kb_i(p),
                        sb['bits'][:, b:b + 1])
                    nc.vector.tensor_scalar_mul(
                        nSsel[:, 128 * b:128 * b + 128], kb_i(p),
                        sb['bits'][:, 6 + b:7 + b])

                def cmul_sym(dstR, dstI, Ar, Ai, nAi, Br, Bi,
                             neg_out_i=False, nAr=None, round_out=False):
                    if round_out and F32R_ADVANCE:
                        dstR = dstR.bitcast(F32R)
                        dstI = dstI.bitcast(F32R)
                    ps = ppA.tile([DIM, 512], F32, tag="mm", name="psCm")
                    nc.tensor.matmul(ps[:, 0:128], Ar, Br, start=True, stop=False)
                    nc.tensor.matmul(ps[:, 0:128], nAi, Bi, start=False, stop=True)
                    if neg_out_i:
                        nc.tensor.matmul(ps[:, 128:256], nAi, Br,
                                         start=True, stop=False)
                        nc.tensor.matmul(ps[:, 128:256], nAr, Bi,
                                         start=False, stop=True)
                    else:
                        nc.tensor.matmul(ps[:, 128:256], Ai, Br,
                                         start=True, stop=False)
                        nc.tensor.matmul(ps[:, 128:256], Ar, Bi,
                                         start=False, stop=True)
                    evac(dstR, ps[:, 0:128])
                    evac(dstI, ps[:, 128:256])

                C12r = ptile("C12r", [DIM, DIM])
                C12i = ptile("C12i", [DIM, DIM])
                cmul_sym(C12r[:], C12i[:],
                         Ssel[:, 0:128], Ssel[:, 128:256], nSsel[:, 0:128],
                         Ssel[:, 256:384], Ssel[:, 384:512])
                nC12i = scratch_neg(C12i[:])
                Qr = ptile("Qr", [DIM, DIM])
                Qi = ptile("Qi", [DIM, DIM])
                cmul_sym(Qr[:], Qi[:], C12r[:], C12i[:], nC12i[:],
                         Ssel[:, 512:640], Ssel[:, 640:768])
                nQr = scratch_neg(Qr[:])
                nQi = scratch_neg(Qi[:])
                # Y = rho0 @ conj(Q): Yr = rho0@Qr ; Yi = -rho0@Qi
                psY = ppA.tile([DIM, 512], F32, tag="mm", name="psY")
                nc.tensor.matmul(psY[:, 0:128], sb['rho0'][:], Qr[:],
                                 start=True, stop=True)
                nc.tensor.matmul(psY[:, 128:256], sb['rho0'][:], Qi[:],
                                 start=True, stop=True)
                Yr = ptile("Yr", [DIM, DIM])
                Yi = ptile("Yi", [DIM, DIM])
                nc.vector.tensor_copy(Yr[:], psY[:, 0:128])
                nc.vector.tensor_scalar_mul(Yi[:], psY[:, 128:256], -1.0)
                # rho = Q @ Y -> working chain j=0 as (Rr, -Ri)
                cmul_sym(STK[:, 0:128], STK[:, 128:256],
                         Qr[:], Qi[:], nQi[:], Yr[:], Yi[:],
                         neg_out_i=True, nAr=nQr[:], round_out=True)

            # ============ stage 5: spawns + q-loop ============
            # STK snapshot-major: slot s = 8j + q at cols 256*s (Rr | -Ri)
            def advance_round(dst_slots, src_slots, p):
                width = len(src_slots)
                w = width * 128
                psZ = None
                for idx, s_src in enumerate(src_slots):
                    lR = STK[:, s_src * 256: s_src * 256 + 128]
                    lN = STK[:, s_src * 256 + 128: s_src * 256 + 256]
                    if idx % 2 == 0:
                        psZ = ppA.tile([DIM, 512], F32, tag="mm", name="psZ")
                    sl = psZ[:, (idx % 2) * 256: (idx % 2) * 256 + 256]
                    nc.tensor.matmul(sl, adv(lR), adv(kbm_ri(p)),
                                     start=True, stop=False)
                    nc.tensor.matmul(sl, adv(lN), adv(kbs_ri(p)),
                                     start=False, stop=True)
                    evac_r(Zr[:, idx * 128:(idx + 1) * 128], sl[:, 0:128])
                    evac_r(Zi[:, idx * 128:(idx + 1) * 128], sl[:, 128:256])
                psR = ppB.tile([DIM, 512], F32, tag="mm2", name="psR")
                psN = ppB.tile([DIM, 512], F32, tag="mm2", name="psN")
                nc.tensor.matmul(psR[:, 0:w], adv(kbq_r(p)), adv(Zr[:, 0:w]),
                                 start=True, stop=False)
                nc.tensor.matmul(psR[:, 0:w], adv(kbn_i(p)), adv(Zi[:, 0:w]),
                                 start=False, stop=True)
                nc.tensor.matmul(psN[:, 0:w], adv(kbn_r(p)), adv(Zi[:, 0:w]),
                                 start=True, stop=False)
                nc.tensor.matmul(psN[:, 0:w], adv(kbn_i(p)), adv(Zr[:, 0:w]),
                                 start=False, stop=True)
                for idx, s_dst in enumerate(dst_slots):
                    evac_r(STK[:, s_dst * 256: s_dst * 256 + 128],
                           psR[:, 128 * idx:128 * idx + 128])
                    evac_r(STK[:, s_dst * 256 + 128: s_dst * 256 + 256],
                           psN[:, 128 * idx:128 * idx + 128])

            with nc.named_scope("evolve"):
                advance_round([16], [0], 256)
                advance_round([8, 24], [0, 16], 128)
                for q in range(1, 8):
                    advance_round([q, 8 + q, 16 + q, 24 + q],
                                  [q - 1, 8 + q - 1, 16 + q - 1, 24 + q - 1],
                                  16)

            # ============ stage 6: Op_r stack ============
            with nc.named_scope("opr"):
                for b4 in range(4):
                    lo = 512 * b4
                    psm = ppA.tile([DIM, 512], F32, tag="mm", name="psm")
                    psn = ppA.tile([DIM, 512], F32, tag="mm", name="psn")
                    nc.tensor.matmul(psm[:, 0:512], opr(sb['opt'][:]),
                                     opr(KLrR[:, lo:lo + 512]),
                                     start=True, stop=True)
                    nc.tensor.matmul(psn[:, 0:512], opr(sb['opt'][:]),
                                     opr(KLiR[:, lo:lo + 512]),
                                     start=True, stop=True)
                    if F32R_OPR:
                        evac(MLr[:, lo:lo + 512].bitcast(F32R), psm[:, 0:512])
                        evac(MLi[:, lo:lo + 512].bitcast(F32R), psn[:, 0:512])
                    else:
                        evac(MLr[:, lo:lo + 512], psm[:, 0:512])
                        evac(MLi[:, lo:lo + 512], psn[:, 0:512])
                for r in range(16):
                    c0 = 128 * r
                    pso = ppA.tile([DIM, 512], F32, tag="mm", name="pso")
                    nc.tensor.matmul(pso[:, 0:128], opr(klr_r(r)),
                                     opr(MLr[:, c0:c0 + 128]),
                                     start=True, stop=False)
                    nc.tensor.matmul(pso[:, 0:128], opr(klr_i(r)),
                                     opr(MLi[:, c0:c0 + 128]),
                                     start=False, stop=True)
                    nc.tensor.matmul(pso[:, 128:256], opr(klr_r(r)),
                                     opr(MLi[:, c0:c0 + 128]),
                                     start=True, stop=False)
                    nc.tensor.matmul(pso[:, 128:256], opr(nkl_i(r)),
                                     opr(MLr[:, c0:c0 + 128]),
                                     start=False, stop=True)
                    evac(OST[:, c0:c0 + 128], pso[:, 0:128])
                    evac(OST[:, 2048 + c0:2048 + c0 + 128], pso[:, 128:256])

            # ============ stage 7: extraction ============
            with nc.named_scope("extract"):
                psPx = ppX.tile([64, 32], F32, tag="px", name="psPx")
                stk4 = STK[:].rearrange("p (s x b) -> p s x b", s=32, x=2)
                ost4 = OST[:].rearrange("p (x r b) -> p x r b", x=2, r=16)
                for b in range(DIM):
                    nc.tensor.matmul(psPx[:], stk4[:, :, :, b], ost4[:, :, :, b],
                                     start=(b == 0), stop=(b == DIM - 1))
                Pxsb = ptile("Pxsb", [64, 32])
                nc.vector.tensor_copy(Pxsb[:], psPx[:])
                pst12 = ppB.tile([32, 64], F32, tag="mm2", name="pst12")
                nc.tensor.matmul(pst12[:, 0:32], sb['selp'][:, 0:32], Pxsb[:],
                                 start=True, stop=True)
                nc.tensor.matmul(pst12[:, 32:64], sb['selp'][:, 32:64], Pxsb[:],
                                 start=True, stop=True)
                T12 = ptile("T12", [32, 64])
                nc.vector.tensor_copy(T12[:], pst12[:])
                fra = ptile("fra", [32, 16])
                fia = ptile("fia", [32, 16])
                fre = ptile("fre", [32, 16])
                fim = ptile("fim", [32, 16])
                nc.vector.tensor_sub(fra[:], T12[:, 0:16], T12[:, 48:64])
                nc.vector.tensor_add(fia[:], T12[:, 16:32], T12[:, 32:48])
                if F32R_FFT:
                    nc.vector.tensor_mul(fre[:].bitcast(F32R), fra[:],
                                         sb['apod'][:])
                    nc.vector.tensor_mul(fim[:].bitcast(F32R), fia[:],
                                         sb['apod'][:])
                else:
                    nc.vector.tensor_mul(fre[:], fra[:], sb['apod'][:])
                    nc.vector.tensor_mul(fim[:], fia[:], sb['apod'][:])
                nc.sync.dma_start(out_fidr[:], fre[:])
                nc.sync.dma_start(out_fidi[:], fim[:])

            # ============ stage 8: partial spectrum (3-stage DFT) ============
            with nc.named_scope("fft"):
                psYa = ppB.tile([16, 512], F32, tag="mm2", name="psYa")
                nc.tensor.matmul(psYa[:], fft(fre[:]), fft(sb['far'][:]),
                                 start=True, stop=False)
                nc.tensor.matmul(psYa[:], fft(fim[:]), fft(sb['fain'][:]),
                                 start=False, stop=True)
                Yfr = ptile("Yfr", [16, 512])
                nc.vector.tensor_copy(Yfr[:], psYa[:])
                psYb = ppB.tile([16, 512], F32, tag="mm2", name="psYb")
                nc.tensor.matmul(psYb[:], fft(fre[:]), fft(sb['fai'][:]),
                                 start=True, stop=False)
                nc.tensor.matmul(psYb[:], fft(fim[:]), fft(sb['far'][:]),
                                 start=False, stop=True)
                Yfi = ptile("Yfi", [16, 512])
                nc.vector.tensor_copy(Yfi[:], psYb[:])
                # Z = Y o TW (elementwise complex, SBUF only)
                za = ptile("za", [16, 512])
                zb = ptile("zb", [16, 512])
                Zfr = ptile("Zfr", [16, 512])
                Zfi = ptile("Zfi", [16, 512])
                nc.vector.tensor_mul(za[:], Yfr[:], sb['twr'][:])
                nc.vector.tensor_mul(zb[:], Yfi[:], sb['twi'][:])
                nc.vector.tensor_sub(
                    Zfr[:].bitcast(F32R) if F32R_FFT else Zfr[:],
                    za[:], zb[:])
                nc.vector.tensor_mul(za[:], Yfr[:], sb['twi'][:])
                nc.vector.tensor_mul(zb[:], Yfi[:], sb['twr'][:])
                nc.vector.tensor_add(
                    Zfi[:].bitcast(F32R) if F32R_FFT else Zfi[:],
                    za[:], zb[:])
                psSa = ppB.tile([16, 512], F32, tag="mm2", name="psSa")
                nc.tensor.matmul(psSa[:], fft(sb['fcr'][:]), fft(Zfr[:]),
                                 start=True, stop=False)
                nc.tensor.matmul(psSa[:], fft(sb['fcin'][:]), fft(Zfi[:]),
                                 start=False, stop=True)
                spR = ptile("spR", [16, 512])
                nc.vector.tensor_copy(spR[:], psSa[:])
                psSb = ppB.tile([16, 512], F32, tag="mm2", name="psSb")
                nc.tensor.matmul(psSb[:], fft(sb['fci'][:]), fft(Zfr[:]),
                                 start=True, stop=False)
                nc.tensor.matmul(psSb[:], fft(sb['fcr'][:]), fft(Zfi[:]),
                                 start=False, stop=True)
                spI = ptile("spI", [16, 512])
                nc.vector.tensor_copy(spI[:], psSb[:])
                nc.sync.dma_start(out_spr[:], spR[:])
                nc.sync.dma_start(out_spi[:], spI[:])

    nc.compile()
    return nc


# ----------------------------------------------------------------------
# entry point
# ----------------------------------------------------------------------
def kernel(h_mat, n_samp):
    from concourse.bass_utils import run_bass_kernel_spmd

    n_samp = int(n_samp)
    assert n_samp == NS, f"kernel compiled for n_samp={NS}, got {n_samp}"
    h_mat = np.asarray(h_mat, dtype=np.float32)
    assert h_mat.shape == (7, 7)

    nc = _build_nc()
    consts = _consts()
    in_maps = []
    for c in range(N_CORES):
        m = dict(consts)
        m.update(_core_inputs(h_mat, c))
        in_maps.append(m)
    res = run_bass_kernel_spmd(nc, in_maps, core_ids=list(range(N_CORES)))

    # gather / unshard
    fid = np.zeros(NS, dtype=np.complex64)
    spec = np.zeros(2 * NS, dtype=np.complex64)
    for c in range(N_CORES):
        r = res.results[c]
        fid[512 * c: 512 * (c + 1)] = (
            r["fidr"].reshape(-1) + 1j * r["fidi"].reshape(-1))
        spec += r["spr"].reshape(-1) + 1j * r["spi"].reshape(-1)

    aq = NS / SW
    time_series = np.linspace(0.0, aq, NS, dtype=np.float32)
    freq_series = np.linspace(-SW / 2, SW / 2, 2 * NS, dtype=np.float32)
    return fid, time_series, spec, freq_series


# revision 11
# speedup vs baseline: 5.1855x; 1.2105x over previous
"""NMR FID simulation kernel for 8 Trainium2 NeuronCores (Bass/Tile).

Math: the reference's sequential scan  rho_{t+1} = P rho_t P^dag,
FID[t] = tr(Op rho_t)  is restructured as a parallel-in-time blocked
Heisenberg evaluation:

    FID[t0 + 16q + r] = tr(Op_r rho_{t0+16q}),   Op_r = K_r^dag Op K_r,
    K_r = P^r  (r < 16),  rho advanced in blocks of 16 via K16 = P^16.

Each of the 8 cores computes 512 consecutive samples (4 rho chains x 8
block-snapshots x 16 in-block samples), then a partial DFT of its own
512 samples (3-stage factorized; fftshift and the per-core global
twiddle folded into constants). Host gathers: concat FID chunks, sum
the 8 partial spectra.

Structure exploited (all verified against the reference numerics):
  - H0 is real symmetric; built on device from h_mat via a
    Walsh-Hadamard factorization of the XOR-structured flip-flop
    couplings (3 matmuls + a few vector ops).
  - P = expm(-i H0 dt) = cos(M) - i sin(M), M = H0*dt, |M| ~ 0.02 ->
    short Taylor series in W = M^2 is exact to fp32.
  - P and all its powers are complex symmetric -> matmul stationary
    operands (lhsT) need no transposes anywhere.
  - rho is Hermitian; stored as (Re rho, -Im rho): rho^T = conj(rho),
    so the stored tiles ARE the lhsT of rho (Re sym, Im antisym).
  - Op = sum_i I+_i is real; rho_0 = U90y IHz U90y^dag = +IHx exactly.
"""

import functools
import numpy as np

N_SPINS = 7
DIM = 128
NS = 4096
DT = 1e-5
B0 = 80.0
T2 = 1.0
SW = 1000.0
N_CORES = 8

# per-stage reduced-precision (float32r) switches for the PE-heavy stages
F32R_ADVANCE = True
F32R_OPR = True
F32R_FFT = True


# ----------------------------------------------------------------------
# host constants (input-independent, same role as reference's _IX etc.)
# ----------------------------------------------------------------------
@functools.lru_cache(maxsize=1)
def _consts():
    f32 = np.float32

    def spin_ops():
        sx = np.array([[0, 1], [1, 0]], dtype=np.complex128) / 2
        sy = np.array([[0, -1j], [1j, 0]], dtype=np.complex128) / 2
        sz = np.array([[1, 0], [0, -1]], dtype=np.complex128) / 2

        def embed(s):
            ops = []
            for i in range(N_SPINS):
                left = np.eye(2**i) if i > 0 else np.array(1.0)
                right = (
                    np.eye(2 ** (N_SPINS - i - 1))
                    if N_SPINS - i - 1 > 0
                    else np.array(1.0)
                )
                ops.append(np.kron(np.kron(left, s), right))
            return np.stack(ops)

        return embed(sx), embed(sy), embed(sz)

    IX, IY, IZ = spin_ops()
    OP = (IX.sum(0) + 1j * IY.sum(0)).real  # total raising operator, real
    RHO0 = IX.sum(0).real                    # U90y IHz U90y^dag == +IHx

    pair = (
        np.einsum('iab,jbc->ijac', IX, IX)
        + np.einsum('iab,jbc->ijac', IY, IY)
        + np.einsum('iab,jbc->ijac', IZ, IZ)
    ).real

    def bitmask(i):
        return 1 << (N_SPINS - 1 - i)

    pairs = [(i, j) for i in range(7) for j in range(i + 1, 7)]
    masks = [bitmask(i) ^ bitmask(j) for (i, j) in pairs]

    BV = np.zeros((DIM, DIM))
    for (i, j), m in zip(pairs, masks):
        FF = pair[i, j].copy()
        np.fill_diagonal(FF, 0.0)
        BV += 2 * np.pi * FF
    DB = np.zeros((28, DIM))
    for i in range(7):
        DB[i] = 2 * np.pi * B0 * np.diag(IZ[i].real)
    for k, (i, j) in enumerate(pairs):
        DB[7 + k] = 2 * np.pi * np.diag(pair[i, j])
    SEL49 = np.zeros((49, 28))
    for i in range(7):
        SEL49[i * 7 + i, i] = 1.0
    for k, (i, j) in enumerate(pairs):
        SEL49[i * 7 + j, 7 + k] = 1.0
    SELM = np.zeros((28, DIM))
    for k, m in enumerate(masks):
        SELM[7 + k, m] = 1.0
    a = np.arange(DIM)
    HAD = np.where(
        np.array([[bin(x & y).count('1') for y in a] for x in a]) % 2 == 0,
        1.0,
        -1.0,
    )

    # extraction row-selectors: T1 = even rows of Px, T2 = odd rows
    SELP = np.zeros((64, 64))
    for s in range(32):
        SELP[2 * s, s] = 1.0
        SELP[2 * s + 1, 32 + s] = 1.0

    s_ = np.arange(32)
    k3 = np.arange(512)
    F512 = np.exp(-2j * np.pi * np.outer(s_, k3) / 512)
    r_ = np.arange(16)
    k4 = np.arange(16)
    F16s = np.exp(-2j * np.pi * np.outer(r_, (k4 + 8) % 16) / 16)  # fftshift folded

    c = {
        'ident': np.eye(DIM),
        'opt': OP.T.copy(),
        'had': HAD,
        'hadn': HAD / DIM,
        'bvdt': BV * DT,   # DT folded: device builds M = H0*DT directly
        'dbdt': DB * DT,
        'sel49': SEL49,
        'selm': SELM,
        'rho0': RHO0,
        'selp': SELP,
        'far': F512.real.copy(),
        'fai': F512.imag.copy(),
        'fcr': F16s.real.copy(),
        'fci': F16s.imag.copy(),
        'fain': -F512.imag,
        'fcin': -F16s.imag,
    }
    return {k: np.ascontiguousarray(v, dtype=f32) for k, v in c.items()}


def _core_inputs(h_mat, core):
    """Per-core config: sharding metadata, apod window, folded twiddles."""
    f32 = np.float32
    t0 = 512 * core
    bits = np.zeros((DIM, 9), f32)
    for b in range(3):
        v = (core >> b) & 1
        bits[:, b] = v
        bits[:, 3 + b] = 1 - v
        bits[:, 6 + b] = -v
    s_idx = np.arange(32)[:, None]
    r_idx = np.arange(16)[None, :]
    apod = np.exp((-DT / T2) * (t0 + 16 * s_idx + r_idx)).astype(f32)
    r_ = np.arange(16)
    k3 = np.arange(512)
    TWc = np.exp(-2j * np.pi * np.outer(r_, k3) / 8192) * np.exp(
        -2j * np.pi * core * k3[None, :] / 16
    )
    return {
        'h49': np.ascontiguousarray(h_mat, dtype=f32).reshape(49, 1),
        'bits': bits,
        'apod': apod,
        'twr': TWc.real.astype(f32).copy(),
        'twi': TWc.imag.astype(f32).copy(),
    }


CONST_SHAPES = {
    'ident': (DIM, DIM), 'opt': (DIM, DIM), 'had': (DIM, DIM),
    'hadn': (DIM, DIM), 'bvdt': (DIM, DIM), 'dbdt': (28, DIM),
    'sel49': (49, 28), 'selm': (28, DIM), 'rho0': (DIM, DIM),
    'selp': (64, 64), 'far': (32, 512), 'fai': (32, 512),
    'fcr': (16, 16), 'fci': (16, 16),
    'fain': (32, 512), 'fcin': (16, 16),
    'h49': (49, 1), 'bits': (DIM, 9), 'apod': (32, 16),
    'twr': (16, 512), 'twi': (16, 512),
}


# ----------------------------------------------------------------------
# bass program
# ----------------------------------------------------------------------
@functools.lru_cache(maxsize=1)
def _build_nc():
    import concourse.mybir as mybir
    import concourse.tile as tile
    from concourse import bacc

    F32 = mybir.dt.float32
    F32R = mybir.dt.float32r
    ALU = mybir.AluOpType

    def adv(ap):   # advance-stage matmul operand view
        if not F32R_ADVANCE:
            return ap
        return ap if ap.dtype == F32R else ap.bitcast(F32R)

    def opr(ap):
        if not F32R_OPR:
            return ap
        return ap if ap.dtype == F32R else ap.bitcast(F32R)

    def fft(ap):
        if not F32R_FFT:
            return ap
        return ap if ap.dtype == F32R else ap.bitcast(F32R)

    nc = bacc.Bacc(None, target_bir_lowering=False)

    F32R_CONSTS = {'opt', 'far', 'fai', 'fain', 'fcr', 'fci', 'fcin'}
    dram = {
        name: nc.dram_tensor(name, list(shape),
                             F32R if name in F32R_CONSTS else F32,
                             kind="ExternalInput")
        for name, shape in CONST_SHAPES.items()
    }
    out_fidr = nc.dram_tensor("fidr", [32, 16], F32, kind="ExternalOutput")
    out_fidi = nc.dram_tensor("fidi", [32, 16], F32, kind="ExternalOutput")
    out_spr = nc.dram_tensor("spr", [16, 512], F32, kind="ExternalOutput")
    out_spi = nc.dram_tensor("spi", [16, 512], F32, kind="ExternalOutput")

    with tile.TileContext(nc) as tc:
        with (
            tc.tile_pool(name="persist", bufs=1) as pp,
            tc.tile_pool(name="psA", bufs=4, space="PSUM") as ppA,   # 4 banks
            tc.tile_pool(name="psB", bufs=2, space="PSUM") as ppB,   # 2 banks
            tc.tile_pool(name="psX", bufs=1, space="PSUM") as ppX,   # 1 bank
        ):
            sb = {}
            for name, shape in CONST_SHAPES.items():
                dt_ = F32R if name in F32R_CONSTS else F32
                sb[name] = pp.tile(list(shape), dt_, tag=name, name=name)
                nc.sync.dma_start(sb[name][:], dram[name][:])

            def ptile(tag, shape):
                t = pp.tile(list(shape), F32, tag=tag, name=tag)
                sb[tag] = t
                return t

            KLr = ptile("KLr", [DIM, 16 * DIM])    # K_r real, r = 0..15
            KLi = ptile("KLi", [DIM, 16 * DIM])
            nKLi = ptile("nKLi", [DIM, 16 * DIM])  # -KLi (rounded)
            KLrR = ptile("KLrR", [DIM, 16 * DIM])   # rounded K_r copies
            KLiR = ptile("KLiR", [DIM, 16 * DIM])
            MLr = ptile("MLr", [DIM, 16 * DIM])    # Op @ K_r
            MLi = ptile("MLi", [DIM, 16 * DIM])
            OST = ptile("OST", [DIM, 2 * 16 * DIM])   # Op_r stack (pp, r, b)
            STK = ptile("STK", [DIM, 32 * 2 * DIM])   # rho snapshots (s, p, b)
            # big powers, blocks p=0..7: K16,K32,K64,K128,K256,K512,A2,A4
            KB = ptile("KB", [DIM, 8 * 256])    # ( Kr |  Ki)
            KBn = ptile("KBn", [DIM, 8 * 256])  # (-Kr | -Ki)
            KBm = ptile("KBm", [DIM, 8 * 256])  # ( Kr | -Ki)  step-1 rhs
            KBs = ptile("KBs", [DIM, 8 * 256])  # ( Ki |  Kr)  step-1 rhs
            KBq = ptile("KBq", [DIM, 8 * 128])  # rounded Kr, step-3 lhsT
            PIDX = {16: 0, 32: 1, 64: 2, 128: 3, 256: 4, 512: 5, 1024: 6, 2048: 7}
            Zr = ptile("Zr", [DIM, 1024])
            Zi = ptile("Zi", [DIM, 1024])

            def kb_r(p): return KB[:, 256 * PIDX[p]: 256 * PIDX[p] + 128]
            def kb_i(p): return KB[:, 256 * PIDX[p] + 128: 256 * PIDX[p] + 256]
            def kbn_r(p): return KBn[:, 256 * PIDX[p]: 256 * PIDX[p] + 128]
            def kbn_i(p): return KBn[:, 256 * PIDX[p] + 128: 256 * PIDX[p] + 256]
            def kbm_ri(p): return KBm[:, 256 * PIDX[p]: 256 * PIDX[p] + 256]
            def kbq_r(p): return KBq[:, 128 * PIDX[p]: 128 * PIDX[p] + 128]
            def kbs_ri(p): return KBs[:, 256 * PIDX[p]: 256 * PIDX[p] + 256]
            def kl_r(r): return KLr[:, 128 * r: 128 * r + 128]
            def kl_i(r): return KLi[:, 128 * r: 128 * r + 128]
            def klr_r(r): return KLrR[:, 128 * r: 128 * r + 128]
            def klr_i(r): return KLiR[:, 128 * r: 128 * r + 128]
            def nkl_i(r): return nKLi[:, 128 * r: 128 * r + 128]

            _evac_flip = [0]

            def evac(dst, src):
                # PSUM -> SBUF, alternating engines for balance
                if _evac_flip[0] % 2 == 0:
                    nc.vector.tensor_copy(dst, src)
                else:
                    nc.scalar.copy(dst, src)
                _evac_flip[0] += 1

            def evac_r(dst, src):
                # PSUM -> SBUF with f32r rounding on write
                d = dst.bitcast(F32R) if F32R_ADVANCE else dst
                if _evac_flip[0] % 2 == 0:
                    nc.vector.tensor_copy(d, src)
                else:
                    nc.scalar.copy(d, src)
                _evac_flip[0] += 1

            def scratch_neg(src):
                t = pp.tile([DIM, DIM], F32, tag="negscr", name="negscr", bufs=4)
                nc.vector.tensor_scalar_mul(t[:], src, -1.0)
                return t

            # ============ stage 1: h_mat -> M = H0*DT (Walsh build) ========
            with nc.named_scope("h0"):
                psT = ppX.tile([DIM, 512], F32, tag="px", name="psT")
                nc.tensor.matmul(psT[0:28, 0:1], sb['sel49'][:], sb['h49'][:],
                                 start=True, stop=True)
                coef = ptile("coef", [28, 1])
                nc.vector.tensor_copy(coef[:], psT[0:28, 0:1])
                nc.tensor.matmul(psT[:, 4:5], sb['selm'][:], coef[:],
                                 start=True, stop=True)
                cmask = ptile("cmask", [DIM, 1])
                nc.vector.tensor_copy(cmask[:], psT[:, 4:5])
                nc.tensor.matmul(psT[:, 8:9], sb['hadn'][:], cmask[:],
                                 start=True, stop=True)
                chat = ptile("chat", [DIM, 1])
                nc.vector.tensor_copy(chat[:], psT[:, 8:9])
                nc.tensor.matmul(psT[:, 12:13], sb['dbdt'][:], coef[:],
                                 start=True, stop=True)
                dvec = ptile("dvec", [DIM, 1])
                nc.vector.tensor_copy(dvec[:], psT[:, 12:13])
                Dc = ptile("Dc", [DIM, DIM])
                nc.vector.tensor_scalar_mul(Dc[:], sb['had'][:], chat[:])
                nc.tensor.matmul(psT[:, 128:256], sb['had'][:], Dc[:],
                                 start=True, stop=True)
                CBsb = ptile("CBsb", [DIM, DIM])
                nc.vector.tensor_copy(CBsb[:], psT[:, 128:256])
                M1 = ptile("M1", [DIM, DIM])
                nc.vector.tensor_mul(M1[:], CBsb[:], sb['bvdt'][:])
                M2 = ptile("M2", [DIM, DIM])
                nc.vector.tensor_scalar_mul(M2[:], sb['ident'][:], dvec[:])
                Mt = ptile("Mt", [DIM, DIM])
                nc.vector.tensor_add(Mt[:], M1[:], M2[:])

            # ============ stage 2: Taylor P = cos(M) - i sin(M) ============
            with nc.named_scope("taylor"):
                ps = ppA.tile([DIM, 512], F32, tag="mm", name="psW")
                nc.tensor.matmul(ps[:, 0:128], Mt[:], Mt[:], start=True, stop=True)
                W = ptile("W", [DIM, DIM])
                evac(W[:], ps[:, 0:128])
                ps = ppA.tile([DIM, 512], F32, tag="mm", name="psW2")
                nc.tensor.matmul(ps[:, 0:128], W[:], W[:], start=True, stop=True)
                W2 = ptile("W2", [DIM, DIM])
                evac(W2[:], ps[:, 0:128])
                ps = ppA.tile([DIM, 512], F32, tag="mm", name="psT3")
                nc.tensor.matmul(ps[:, 0:128], W[:], W2[:], start=True, stop=True)
                T3 = ptile("T3", [DIM, DIM])
                evac(T3[:], ps[:, 0:128])
                # K1 real = I - W/2 + W2/24 - T3/720
                t1 = ptile("tay1", [DIM, DIM])
                nc.vector.scalar_tensor_tensor(
                    t1[:], W[:], -0.5, sb['ident'][:], ALU.mult, ALU.add)
                t2 = ptile("tay2", [DIM, DIM])
                nc.vector.scalar_tensor_tensor(
                    t2[:], W2[:], 1.0 / 24, t1[:], ALU.mult, ALU.add)
                nc.vector.scalar_tensor_tensor(
                    KLr[:, 128:256], T3[:], -1.0 / 720, t2[:], ALU.mult, ALU.add)
                # negE = W/6 - I - W2/120 + T3/5040 ; K1 imag = M @ negE
                t3 = ptile("tay3", [DIM, DIM])
                nc.vector.scalar_tensor_tensor(
                    t3[:], W[:], 1.0 / 6, sb['ident'][:], ALU.mult, ALU.subtract)
                t4 = ptile("tay4", [DIM, DIM])
                nc.vector.scalar_tensor_tensor(
                    t4[:], W2[:], -1.0 / 120, t3[:], ALU.mult, ALU.add)
                negE = ptile("negE", [DIM, DIM])
                nc.vector.scalar_tensor_tensor(
                    negE[:], T3[:], 1.0 / 5040, t4[:], ALU.mult, ALU.add)
                ps = ppA.tile([DIM, 512], F32, tag="mm", name="psKi")
                nc.tensor.matmul(ps[:, 0:128], Mt[:], negE[:], start=True, stop=True)
                evac(KLi[:, 128:256], ps[:, 0:128])
                nc.vector.tensor_copy(KLr[:, 0:128], sb['ident'][:])
                nc.vector.memset(KLi[:, 0:128], 0.0)

            # ============ stage 3: ladder ============
            def rnd(ap):
                return ap.bitcast(F32R) if F32R_ADVANCE else ap

            def arrange_kb(p, mixed):
                """Rounded f32r arrangements of block p for the advance stage."""
                nc.vector.tensor_scalar_mul(rnd(kbn_r(p)), kb_r(p), -1.0)
                nc.vector.tensor_scalar_mul(rnd(kbn_i(p)), kb_i(p), -1.0)
                if mixed:
                    nc.scalar.copy(rnd(kbq_r(p)), kb_r(p))
                    ri = kbm_ri(p)
                    nc.vector.tensor_copy(rnd(ri[:, 0:128]), kb_r(p))
                    nc.vector.tensor_scalar_mul(rnd(ri[:, 128:256]),
                                                kb_i(p), -1.0)
                    ri = kbs_ri(p)
                    nc.scalar.copy(rnd(ri[:, 0:128]), kb_i(p))
                    nc.scalar.copy(rnd(ri[:, 128:256]), kb_r(p))

            with nc.named_scope("ladder"):
                for n in [1, 2, 4, 8]:
                    lo, hi = n + 1, min(2 * n, 16)
                    w = (hi - lo + 1) * 128
                    nki = scratch_neg(kl_i(n))
                    for base in range(0, w, 512):
                        cw = min(512, w - base)
                        boff = 128 + base
                        psR = ppA.tile([DIM, 512], F32, tag="mm", name="psLr")
                        psI = ppA.tile([DIM, 512], F32, tag="mm", name="psLi")
                        nc.tensor.matmul(psR[:, 0:cw], kl_r(n),
                                         KLr[:, boff:boff + cw],
                                         start=True, stop=False)
                        nc.tensor.matmul(psR[:, 0:cw], nki[:],
                                         KLi[:, boff:boff + cw],
                                         start=False, stop=True)
                        nc.tensor.matmul(psI[:, 0:cw], kl_i(n),
                                         KLr[:, boff:boff + cw],
                                         start=True, stop=False)
                        nc.tensor.matmul(psI[:, 0:cw], kl_r(n),
                                         KLi[:, boff:boff + cw],
                                         start=False, stop=True)
                        doff = 128 * lo + base
                        dw = min(cw, 2048 - doff) if doff < 2048 else 0
                        if dw > 0:
                            evac(KLr[:, doff:doff + dw], psR[:, 0:dw])
                            evac(KLi[:, doff:doff + dw], psI[:, 0:dw])
                        if dw < cw:      # tail target is K16 -> KB block 0
                            evac(kb_r(16), psR[:, dw:cw])
                            evac(kb_i(16), psI[:, dw:cw])
                arrange_kb(16, mixed=True)
                nknext = scratch_neg(kb_i(16))
                for p in [32, 64, 128, 256, 512, 1024, 2048]:
                    h = p // 2
                    nkh = nknext
                    ps = ppA.tile([DIM, 512], F32, tag="mm", name="psSq")
                    nc.tensor.matmul(ps[:, 0:128], kb_r(h), kb_r(h),
                                     start=True, stop=False)
                    nc.tensor.matmul(ps[:, 0:128], nkh[:], kb_i(h),
                                     start=False, stop=True)
                    nc.tensor.matmul(ps[:, 128:256], kb_i(h), kb_r(h),
                                     start=True, stop=False)
                    nc.tensor.matmul(ps[:, 128:256], kb_r(h), kb_i(h),
                                     start=False, stop=True)
                    nc.vector.tensor_copy(kb_r(p), ps[:, 0:128])
                    nc.scalar.copy(kb_i(p), ps[:, 128:256])
                    if p != 2048:
                        nknext = pp.tile([DIM, DIM], F32, tag="negscr",
                                         name="negscr", bufs=4)
                        nc.vector.tensor_scalar_mul(nknext[:],
                                                    ps[:, 128:256], -1.0)
                    if p in (64, 128, 256):
                        arrange_kb(p, mixed=True)
                if F32R_OPR:
                    nc.scalar.mul(nKLi[:].bitcast(F32R), KLi[:], -1.0)
                    nc.vector.tensor_copy(KLrR[:].bitcast(F32R), KLr[:])
                    nc.scalar.copy(KLiR[:].bitcast(F32R), KLi[:])
                else:
                    nc.scalar.mul(nKLi[:], KLi[:], -1.0)
                    nc.vector.tensor_copy(KLrR[:], KLr[:])
                    nc.scalar.copy(KLiR[:], KLi[:])

            # ============ stage 4: per-core Q & rho start ============
            with nc.named_scope("start"):
                Ssel = ptile("Ssel", [DIM, 6 * 128])    # (Sr|Si) x 3
                nSsel = ptile("nSsel", [DIM, 3 * 128])  # -Si x 3
                for b, p in enumerate([512, 1024, 2048]):
                    tb = ptile("selt%d" % b, [DIM, DIM])
                    nc.vector.tensor_scalar_mul(
                        tb[:], sb['ident'][:], sb['bits'][:, 3 + b:4 + b])
                    nc.vector.scalar_tensor_tensor(
                        Ssel[:, 256 * b:256 * b + 128], kb_r(p),
                        sb['bits'][:, b:b + 1], tb[:], ALU.mult, ALU.add)
                    nc.vector.tensor_scalar_mul(
                        Ssel[:, 256 * b + 128:256 * b + 256], kb_i(p),
                        sb['bits'][:, b:b + 1])
                    nc.vector.tensor_scalar_mul(
                        nSsel[:, 128 * b:128 * b + 128], kb_i(p),
                        sb['bits'][:, 6 + b:7 + b])

                def cmul_sym(dstR, dstI, Ar, Ai, nAi, Br, Bi,
                             neg_out_i=False, nAr=None, round_out=False):
                    if round_out and F32R_ADVANCE:
                        dstR = dstR.bitcast(F32R)
                        dstI = dstI.bitcast(F32R)
                    ps = ppA.tile([DIM, 512], F32, tag="mm", name="psCm")
                    nc.tensor.matmul(ps[:, 0:128], Ar, Br, start=True, stop=False)
                    nc.tensor.matmul(ps[:, 0:128], nAi, Bi, start=False, stop=True)
                    if neg_out_i:
                        nc.tensor.matmul(ps[:, 128:256], nAi, Br,
                                         start=True, stop=False)
                        nc.tensor.matmul(ps[:, 128:256], nAr, Bi,
                                         start=False, stop=True)
                    else:
                        nc.tensor.matmul(ps[:, 128:256], Ai, Br,
                                         start=True, stop=False)
                        nc.tensor.matmul(ps[:, 128:256], Ar, Bi,
                                         start=False, stop=True)
                    evac(dstR, ps[:, 0:128])
                    evac(dstI, ps[:, 128:256])

                C12r = ptile("C12r", [DIM, DIM])
                C12i = ptile("C12i", [DIM, DIM])
                cmul_sym(C12r[:], C12i[:],
                         Ssel[:, 0:128], Ssel[:, 128:256], nSsel[:, 0:128],
                         Ssel[:, 256:384], Ssel[:, 384:512])
                nC12i = scratch_neg(C12i[:])
                Qr = ptile("Qr", [DIM, DIM])
                Qi = ptile("Qi", [DIM, DIM])
                cmul_sym(Qr[:], Qi[:], C12r[:], C12i[:], nC12i[:],
                         Ssel[:, 512:640], Ssel[:, 640:768])
                nQr = scratch_neg(Qr[:])
                nQi = scratch_neg(Qi[:])
                # Y = rho0 @ conj(Q): Yr = rho0@Qr ; Yi = -rho0@Qi
                psY = ppA.tile([DIM, 512], F32, tag="mm", name="psY")
                nc.tensor.matmul(psY[:, 0:128], sb['rho0'][:], Qr[:],
                                 start=True, stop=True)
                nc.tensor.matmul(psY[:, 128:256], sb['rho0'][:], Qi[:],
                                 start=True, stop=True)
                Yr = ptile("Yr", [DIM, DIM])
                Yi = ptile("Yi", [DIM, DIM])
                nc.vector.tensor_copy(Yr[:], psY[:, 0:128])
                nc.vector.tensor_scalar_mul(Yi[:], psY[:, 128:256], -1.0)
                # rho = Q @ Y -> working chain j=0 as (Rr, -Ri)
                cmul_sym(STK[:, 0:128], STK[:, 128:256],
                         Qr[:], Qi[:], nQi[:], Yr[:], Yi[:],
                         neg_out_i=True, nAr=nQr[:], round_out=True)

            # ============ stage 5: spawns + q-loop ============
            # STK snapshot-major: slot s = 8j + q at cols 256*s (Rr | -Ri)
            def advance_round(dst_slots, src_slots, p):
                width = len(src_slots)
                w = width * 128
                psZ = None
                for idx, s_src in enumerate(src_slots):
                    lR = STK[:, s_src * 256: s_src * 256 + 128]
                    lN = STK[:, s_src * 256 + 128: s_src * 256 + 256]
                    if idx % 2 == 0:
                        psZ = ppA.tile([DIM, 512], F32, tag="mm", name="psZ")
                    sl = psZ[:, (idx % 2) * 256: (idx % 2) * 256 + 256]
                    nc.tensor.matmul(sl, adv(lR), adv(kbm_ri(p)),
                                     start=True, stop=False)
                    nc.tensor.matmul(sl, adv(lN), adv(kbs_ri(p)),
                                     start=False, stop=True)
                    evac_r(Zr[:, idx * 128:(idx + 1) * 128], sl[:, 0:128])
                    evac_r(Zi[:, idx * 128:(idx + 1) * 128], sl[:, 128:256])
                for h0_ in range(0, w, 512):
                    hw_ = min(512, w - h0_)
                    psR = ppB.tile([DIM, 512], F32, tag="mm2", name="psR")
                    psN = ppB.tile([DIM, 512], F32, tag="mm2", name="psN")
                    nc.tensor.matmul(psR[:, 0:hw_], adv(kbq_r(p)),
                                     adv(Zr[:, h0_:h0_ + hw_]),
                                     start=True, stop=False)
                    nc.tensor.matmul(psR[:, 0:hw_], adv(kbn_i(p)),
                                     adv(Zi[:, h0_:h0_ + hw_]),
                                     start=False, stop=True)
                    nc.tensor.matmul(psN[:, 0:hw_], adv(kbn_r(p)),
                                     adv(Zi[:, h0_:h0_ + hw_]),
                                     start=True, stop=False)
                    nc.tensor.matmul(psN[:, 0:hw_], adv(kbn_i(p)),
                                     adv(Zr[:, h0_:h0_ + hw_]),
                                     start=False, stop=True)
                    for k_ in range(hw_ // 128):
                        s_dst = dst_slots[h0_ // 128 + k_]
                        evac_r(STK[:, s_dst * 256: s_dst * 256 + 128],
                               psR[:, 128 * k_:128 * k_ + 128])
                        evac_r(STK[:, s_dst * 256 + 128: s_dst * 256 + 256],
                               psN[:, 128 * k_:128 * k_ + 128])

            with nc.named_scope("evolve"):
                # chain j covers t0 + 64*j; slot s = 4*j + q (q < 4)
                advance_round([16], [0], 256)                  # j4 = j0+256
                advance_round([8, 24], [0, 16], 128)           # j2, j6
                advance_round([4, 12, 20, 28], [0, 8, 16, 24], 64)   # j1,3,5,7
                for q in range(1, 4):
                    advance_round([4 * j + q for j in range(8)],
                                  [4 * j + q - 1 for j in range(8)],
                                  16)

            # ============ stage 6: Op_r stack ============
            with nc.named_scope("opr"):
                for b4 in range(4):
                    lo = 512 * b4
                    psm = ppA.tile([DIM, 512], F32, tag="mm", name="psm")
                    psn = ppA.tile([DIM, 512], F32, tag="mm", name="psn")
                    nc.tensor.matmul(psm[:, 0:512], opr(sb['opt'][:]),
                                     opr(KLrR[:, lo:lo + 512]),
                                     start=True, stop=True)
                    nc.tensor.matmul(psn[:, 0:512], opr(sb['opt'][:]),
                                     opr(KLiR[:, lo:lo + 512]),
                                     start=True, stop=True)
                    if F32R_OPR:
                        evac(MLr[:, lo:lo + 512].bitcast(F32R), psm[:, 0:512])
                        evac(MLi[:, lo:lo + 512].bitcast(F32R), psn[:, 0:512])
                    else:
                        evac(MLr[:, lo:lo + 512], psm[:, 0:512])
                        evac(MLi[:, lo:lo + 512], psn[:, 0:512])
                for r in range(16):
                    c0 = 128 * r
                    pso = ppA.tile([DIM, 512], F32, tag="mm", name="pso")
                    nc.tensor.matmul(pso[:, 0:128], opr(klr_r(r)),
                                     opr(MLr[:, c0:c0 + 128]),
                                     start=True, stop=False)
                    nc.tensor.matmul(pso[:, 0:128], opr(klr_i(r)),
                                     opr(MLi[:, c0:c0 + 128]),
                                     start=False, stop=True)
                    nc.tensor.matmul(pso[:, 128:256], opr(klr_r(r)),
                                     opr(MLi[:, c0:c0 + 128]),
                                     start=True, stop=False)
                    nc.tensor.matmul(pso[:, 128:256], opr(nkl_i(r)),
                                     opr(MLr[:, c0:c0 + 128]),
                                     start=False, stop=True)
                    evac_r(OST[:, c0:c0 + 128], pso[:, 0:128])
                    evac_r(OST[:, 2048 + c0:2048 + c0 + 128], pso[:, 128:256])

            # ============ stage 7: extraction ============
            with nc.named_scope("extract"):
                psPx = ppX.tile([64, 32], F32, tag="px", name="psPx")
                stk4 = STK[:].rearrange("p (s x b) -> p s x b", s=32, x=2)
                ost4 = OST[:].rearrange("p (x r b) -> p x r b", x=2, r=16)
                for b in range(DIM):
                    nc.tensor.matmul(psPx[:], adv(stk4[:, :, :, b]),
                                     adv(ost4[:, :, :, b]),
                                     start=(b == 0), stop=(b == DIM - 1))
                Pxsb = ptile("Pxsb", [64, 32])
                nc.vector.tensor_copy(Pxsb[:], psPx[:])
                pst12 = ppB.tile([32, 64], F32, tag="mm2", name="pst12")
                nc.tensor.matmul(pst12[:, 0:32], sb['selp'][:, 0:32], Pxsb[:],
                                 start=True, stop=True)
                nc.tensor.matmul(pst12[:, 32:64], sb['selp'][:, 32:64], Pxsb[:],
                                 start=True, stop=True)
                T12 = ptile("T12", [32, 64])
                nc.vector.tensor_copy(T12[:], pst12[:])
                fra = ptile("fra", [32, 16])
                fia = ptile("fia", [32, 16])
                fre = ptile("fre", [32, 16])
                fim = ptile("fim", [32, 16])
                nc.vector.tensor_sub(fra[:], T12[:, 0:16], T12[:, 48:64])
                nc.vector.tensor_add(fia[:], T12[:, 16:32], T12[:, 32:48])
                if F32R_FFT:
                    nc.vector.tensor_mul(fre[:].bitcast(F32R), fra[:],
                                         sb['apod'][:])
                    nc.vector.tensor_mul(fim[:].bitcast(F32R), fia[:],
                                         sb['apod'][:])
                else:
                    nc.vector.tensor_mul(fre[:], fra[:], sb['apod'][:])
                    nc.vector.tensor_mul(fim[:], fia[:], sb['apod'][:])
                nc.sync.dma_start(out_fidr[:], fre[:])
                nc.sync.dma_start(out_fidi[:], fim[:])

            # ============ stage 8: partial spectrum (3-stage DFT) ============
            with nc.named_scope("fft"):
                psYa = ppB.tile([16, 512], F32, tag="mm2", name="psYa")
                nc.tensor.matmul(psYa[:], fft(fre[:]), fft(sb['far'][:]),
                                 start=True, stop=False)
                nc.tensor.matmul(psYa[:], fft(fim[:]), fft(sb['fain'][:]),
                                 start=False, stop=True)
                Yfr = ptile("Yfr", [16, 512])
                nc.vector.tensor_copy(Yfr[:], psYa[:])
                psYb = ppB.tile([16, 512], F32, tag="mm2", name="psYb")
                nc.tensor.matmul(psYb[:], fft(fre[:]), fft(sb['fai'][:]),
                                 start=True, stop=False)
                nc.tensor.matmul(psYb[:], fft(fim[:]), fft(sb['far'][:]),
                                 start=False, stop=True)
                Yfi = ptile("Yfi", [16, 512])
                nc.vector.tensor_copy(Yfi[:], psYb[:])
                # Z = Y o TW (elementwise complex, SBUF only)
                za = ptile("za", [16, 512])
                zb = ptile("zb", [16, 512])
                Zfr = ptile("Zfr", [16, 512])
                Zfi = ptile("Zfi", [16, 512])
                nc.vector.tensor_mul(za[:], Yfr[:], sb['twr'][:])
                nc.vector.tensor_mul(zb[:], Yfi[:], sb['twi'][:])
                nc.vector.tensor_sub(
                    Zfr[:].bitcast(F32R) if F32R_FFT else Zfr[:],
                    za[:], zb[:])
                nc.vector.tensor_mul(za[:], Yfr[:], sb['twi'][:])
                nc.vector.tensor_mul(zb[:], Yfi[:], sb['twr'][:])
                nc.vector.tensor_add(
                    Zfi[:].bitcast(F32R) if F32R_FFT else Zfi[:],
                    za[:], zb[:])
                psSa = ppB.tile([16, 512], F32, tag="mm2", name="psSa")
                nc.tensor.matmul(psSa[:], fft(sb['fcr'][:]), fft(Zfr[:]),
                                 start=True, stop=False)
                nc.tensor.matmul(psSa[:], fft(sb['fcin'][:]), fft(Zfi[:]),
                                 start=False, stop=True)
                spR = ptile("spR", [16, 512])
                nc.vector.tensor_copy(spR[:], psSa[:])
                psSb = ppB.tile([16, 512], F32, tag="mm2", name="psSb")
                nc.tensor.matmul(psSb[:], fft(sb['fci'][:]), fft(Zfr[:]),
                                 start=True, stop=False)
                nc.tensor.matmul(psSb[:], fft(sb['fcr'][:]), fft(Zfi[:]),
                                 start=False, stop=True)
                spI = ptile("spI", [16, 512])
                nc.vector.tensor_copy(spI[:], psSb[:])
                nc.sync.dma_start(out_spr[:], spR[:])
                nc.sync.dma_start(out_spi[:], spI[:])

    nc.compile()
    return nc


# ----------------------------------------------------------------------
# entry point
# ----------------------------------------------------------------------
def kernel(h_mat, n_samp):
    from concourse.bass_utils import run_bass_kernel_spmd

    n_samp = int(n_samp)
    assert n_samp == NS, f"kernel compiled for n_samp={NS}, got {n_samp}"
    h_mat = np.asarray(h_mat, dtype=np.float32)
    assert h_mat.shape == (7, 7)

    nc = _build_nc()
    consts = _consts()
    in_maps = []
    for c in range(N_CORES):
        m = dict(consts)
        m.update(_core_inputs(h_mat, c))
        in_maps.append(m)
    res = run_bass_kernel_spmd(nc, in_maps, core_ids=list(range(N_CORES)))

    # gather / unshard
    fid = np.zeros(NS, dtype=np.complex64)
    spec = np.zeros(2 * NS, dtype=np.complex64)
    for c in range(N_CORES):
        r = res.results[c]
        fid[512 * c: 512 * (c + 1)] = (
            r["fidr"].reshape(-1) + 1j * r["fidi"].reshape(-1))
        spec += r["spr"].reshape(-1) + 1j * r["spi"].reshape(-1)

    aq = NS / SW
    time_series = np.linspace(0.0, aq, NS, dtype=np.float32)
    freq_series = np.linspace(-SW / 2, SW / 2, 2 * NS, dtype=np.float32)
    return fid, time_series, spec, freq_series


# revision 12
# speedup vs baseline: 5.6692x; 1.0933x over previous
"""NMR FID simulation kernel for 8 Trainium2 NeuronCores (Bass/Tile).

Math: the reference's sequential scan  rho_{t+1} = P rho_t P^dag,
FID[t] = tr(Op rho_t)  is restructured as a parallel-in-time blocked
Heisenberg evaluation:

    FID[t0 + 16q + r] = tr(Op_r rho_{t0+16q}),   Op_r = K_r^dag Op K_r,
    K_r = P^r  (r < 16),  rho advanced in blocks of 16 via K16 = P^16.

Each of the 8 cores computes 512 consecutive samples (4 rho chains x 8
block-snapshots x 16 in-block samples), then a partial DFT of its own
512 samples (3-stage factorized; fftshift and the per-core global
twiddle folded into constants). Host gathers: concat FID chunks, sum
the 8 partial spectra.

Structure exploited (all verified against the reference numerics):
  - H0 is real symmetric; built on device from h_mat via a
    Walsh-Hadamard factorization of the XOR-structured flip-flop
    couplings (3 matmuls + a few vector ops).
  - P = expm(-i H0 dt) = cos(M) - i sin(M), M = H0*dt, |M| ~ 0.02 ->
    short Taylor series in W = M^2 is exact to fp32.
  - P and all its powers are complex symmetric -> matmul stationary
    operands (lhsT) need no transposes anywhere.
  - rho is Hermitian; stored as (Re rho, -Im rho): rho^T = conj(rho),
    so the stored tiles ARE the lhsT of rho (Re sym, Im antisym).
  - Op = sum_i I+_i is real; rho_0 = U90y IHz U90y^dag = +IHx exactly.
"""

import functools
import numpy as np

N_SPINS = 7
DIM = 128
NS = 4096
DT = 1e-5
B0 = 80.0
T2 = 1.0
SW = 1000.0
N_CORES = 8

# per-stage reduced-precision (float32r) switches for the PE-heavy stages
F32R_ADVANCE = True
F32R_OPR = True
F32R_FFT = True


# ----------------------------------------------------------------------
# host constants (input-independent, same role as reference's _IX etc.)
# ----------------------------------------------------------------------
@functools.lru_cache(maxsize=1)
def _consts():
    f32 = np.float32

    def spin_ops():
        sx = np.array([[0, 1], [1, 0]], dtype=np.complex128) / 2
        sy = np.array([[0, -1j], [1j, 0]], dtype=np.complex128) / 2
        sz = np.array([[1, 0], [0, -1]], dtype=np.complex128) / 2

        def embed(s):
            ops = []
            for i in range(N_SPINS):
                left = np.eye(2**i) if i > 0 else np.array(1.0)
                right = (
                    np.eye(2 ** (N_SPINS - i - 1))
                    if N_SPINS - i - 1 > 0
                    else np.array(1.0)
                )
                ops.append(np.kron(np.kron(left, s), right))
            return np.stack(ops)

        return embed(sx), embed(sy), embed(sz)

    IX, IY, IZ = spin_ops()
    OP = (IX.sum(0) + 1j * IY.sum(0)).real  # total raising operator, real
    RHO0 = IX.sum(0).real                    # U90y IHz U90y^dag == +IHx

    pair = (
        np.einsum('iab,jbc->ijac', IX, IX)
        + np.einsum('iab,jbc->ijac', IY, IY)
        + np.einsum('iab,jbc->ijac', IZ, IZ)
    ).real

    def bitmask(i):
        return 1 << (N_SPINS - 1 - i)

    pairs = [(i, j) for i in range(7) for j in range(i + 1, 7)]
    masks = [bitmask(i) ^ bitmask(j) for (i, j) in pairs]

    BV = np.zeros((DIM, DIM))
    for (i, j), m in zip(pairs, masks):
        FF = pair[i, j].copy()
        np.fill_diagonal(FF, 0.0)
        BV += 2 * np.pi * FF
    DB = np.zeros((28, DIM))
    for i in range(7):
        DB[i] = 2 * np.pi * B0 * np.diag(IZ[i].real)
    for k, (i, j) in enumerate(pairs):
        DB[7 + k] = 2 * np.pi * np.diag(pair[i, j])
    SEL49 = np.zeros((49, 28))
    for i in range(7):
        SEL49[i * 7 + i, i] = 1.0
    for k, (i, j) in enumerate(pairs):
        SEL49[i * 7 + j, 7 + k] = 1.0
    SELM = np.zeros((28, DIM))
    for k, m in enumerate(masks):
        SELM[7 + k, m] = 1.0
    a = np.arange(DIM)
    HAD = np.where(
        np.array([[bin(x & y).count('1') for y in a] for x in a]) % 2 == 0,
        1.0,
        -1.0,
    )

    # extraction row-selectors: T1 = even rows of Px, T2 = odd rows
    SELP = np.zeros((64, 64))
    for s in range(32):
        SELP[2 * s, s] = 1.0
        SELP[2 * s + 1, 32 + s] = 1.0

    s_ = np.arange(32)
    k3 = np.arange(512)
    F512 = np.exp(-2j * np.pi * np.outer(s_, k3) / 512)
    r_ = np.arange(16)
    k4 = np.arange(16)
    F16s = np.exp(-2j * np.pi * np.outer(r_, (k4 + 8) % 16) / 16)  # fftshift folded

    c = {
        'ident': np.eye(DIM),
        'opt': OP.T.copy(),
        'had': HAD,
        'bvdt': BV * DT,   # DT folded: device builds M = H0*DT directly
        'chT': SEL49 @ SELM @ (HAD / DIM),
        'dvT': SEL49 @ (DB * DT),
        'rho0': RHO0,
        'selp': SELP,
        'far': F512.real.copy(),
        'fai': F512.imag.copy(),
        'fcr': F16s.real.copy(),
        'fci': F16s.imag.copy(),
        'fain': -F512.imag,
        'fcin': -F16s.imag,
    }
    return {k: np.ascontiguousarray(v, dtype=f32) for k, v in c.items()}


def _core_inputs(h_mat, core):
    """Per-core config: sharding metadata, apod window, folded twiddles."""
    f32 = np.float32
    t0 = 512 * core
    bits = np.zeros((DIM, 9), f32)
    for b in range(3):
        v = (core >> b) & 1
        bits[:, b] = v
        bits[:, 3 + b] = 1 - v
        bits[:, 6 + b] = -v
    s_idx = np.arange(32)[:, None]
    r_idx = np.arange(16)[None, :]
    apod = np.exp((-DT / T2) * (t0 + 16 * s_idx + r_idx)).astype(f32)
    r_ = np.arange(16)
    k3 = np.arange(512)
    TWc = np.exp(-2j * np.pi * np.outer(r_, k3) / 8192) * np.exp(
        -2j * np.pi * core * k3[None, :] / 16
    )
    return {
        'h49': np.ascontiguousarray(h_mat, dtype=f32).reshape(49, 1),
        'bits': bits,
        'apod': apod,
        'twr': TWc.real.astype(f32).copy(),
        'twi': TWc.imag.astype(f32).copy(),
    }


CONST_SHAPES = {
    'ident': (DIM, DIM), 'opt': (DIM, DIM), 'had': (DIM, DIM),
    'bvdt': (DIM, DIM),
    'chT': (49, DIM), 'dvT': (49, DIM), 'rho0': (DIM, DIM),
    'selp': (64, 64), 'far': (32, 512), 'fai': (32, 512),
    'fcr': (16, 16), 'fci': (16, 16),
    'fain': (32, 512), 'fcin': (16, 16),
    'h49': (49, 1), 'bits': (DIM, 9), 'apod': (32, 16),
    'twr': (16, 512), 'twi': (16, 512),
}


# ----------------------------------------------------------------------
# bass program
# ----------------------------------------------------------------------
@functools.lru_cache(maxsize=1)
def _build_nc():
    import concourse.mybir as mybir
    import concourse.tile as tile
    from concourse import bacc

    F32 = mybir.dt.float32
    F32R = mybir.dt.float32r
    ALU = mybir.AluOpType

    def adv(ap):   # advance-stage matmul operand view
        if not F32R_ADVANCE:
            return ap
        return ap if ap.dtype == F32R else ap.bitcast(F32R)

    def opr(ap):
        if not F32R_OPR:
            return ap
        return ap if ap.dtype == F32R else ap.bitcast(F32R)

    def fft(ap):
        if not F32R_FFT:
            return ap
        return ap if ap.dtype == F32R else ap.bitcast(F32R)

    nc = bacc.Bacc(None, target_bir_lowering=False)

    F32R_CONSTS = {'opt', 'far', 'fai', 'fain', 'fcr', 'fci', 'fcin'}
    dram = {
        name: nc.dram_tensor(name, list(shape),
                             F32R if name in F32R_CONSTS else F32,
                             kind="ExternalInput")
        for name, shape in CONST_SHAPES.items()
    }
    out_fidr = nc.dram_tensor("fidr", [32, 16], F32, kind="ExternalOutput")
    out_fidi = nc.dram_tensor("fidi", [32, 16], F32, kind="ExternalOutput")
    out_spr = nc.dram_tensor("spr", [16, 512], F32, kind="ExternalOutput")
    out_spi = nc.dram_tensor("spi", [16, 512], F32, kind="ExternalOutput")

    with tile.TileContext(nc) as tc:
        with (
            tc.tile_pool(name="persist", bufs=1) as pp,
            tc.tile_pool(name="psA", bufs=4, space="PSUM") as ppA,   # 4 banks
            tc.tile_pool(name="psB", bufs=2, space="PSUM") as ppB,   # 2 banks
            tc.tile_pool(name="psX", bufs=1, space="PSUM") as ppX,   # 1 bank
        ):
            sb = {}
            _order = ['h49', 'chT', 'dvT', 'had', 'bvdt', 'ident', 'opt',
                      'rho0', 'bits']
            _names = _order + [n for n in CONST_SHAPES if n not in _order]
            for name in _names:
                shape = CONST_SHAPES[name]
                dt_ = F32R if name in F32R_CONSTS else F32
                sb[name] = pp.tile(list(shape), dt_, tag=name, name=name)
                nc.sync.dma_start(sb[name][:], dram[name][:])

            def ptile(tag, shape):
                t = pp.tile(list(shape), F32, tag=tag, name=tag)
                sb[tag] = t
                return t

            KLr = ptile("KLr", [DIM, 16 * DIM])    # K_r real, r = 0..15
            KLi = ptile("KLi", [DIM, 16 * DIM])
            KLrR = ptile("KLrR", [DIM, 16 * DIM])   # rounded K_r copies
            KLiR = ptile("KLiR", [DIM, 16 * DIM])
            MLri = ptile("MLri", [DIM, 16 * 256])  # (Mr_r |  Mi_r) blocks
            MLsw = ptile("MLsw", [DIM, 16 * 256])  # (Mi_r | -Mr_r) blocks
            OST = ptile("OST", [DIM, 2 * 16 * DIM])   # Op_r stack (pp, r, b)
            STK = ptile("STK", [DIM, 32 * 2 * DIM])   # rho snapshots (s, p, b)
            # big powers, blocks p=0..7: K16,K32,K64,K128,K256,K512,A2,A4
            KB = ptile("KB", [DIM, 8 * 256])    # ( Kr |  Ki)
            KBn = ptile("KBn", [DIM, 8 * 256])  # (-Kr | -Ki)
            KBm = ptile("KBm", [DIM, 8 * 256])  # ( Kr | -Ki)  step-1 rhs
            KBs = ptile("KBs", [DIM, 8 * 256])  # ( Ki |  Kr)  step-1 rhs
            KBq = ptile("KBq", [DIM, 8 * 128])  # rounded Kr, step-3 lhsT
            PIDX = {16: 0, 32: 1, 64: 2, 128: 3, 256: 4, 512: 5, 1024: 6, 2048: 7}
            Zr = ptile("Zr", [DIM, 1024])
            Zi = ptile("Zi", [DIM, 1024])

            def kb_r(p): return KB[:, 256 * PIDX[p]: 256 * PIDX[p] + 128]
            def kb_i(p): return KB[:, 256 * PIDX[p] + 128: 256 * PIDX[p] + 256]
            def kbn_r(p): return KBn[:, 256 * PIDX[p]: 256 * PIDX[p] + 128]
            def kbn_i(p): return KBn[:, 256 * PIDX[p] + 128: 256 * PIDX[p] + 256]
            def kbm_ri(p): return KBm[:, 256 * PIDX[p]: 256 * PIDX[p] + 256]
            def kbq_r(p): return KBq[:, 128 * PIDX[p]: 128 * PIDX[p] + 128]
            def kbs_ri(p): return KBs[:, 256 * PIDX[p]: 256 * PIDX[p] + 256]
            def kl_r(r): return KLr[:, 128 * r: 128 * r + 128]
            def kl_i(r): return KLi[:, 128 * r: 128 * r + 128]
            def klr_r(r): return KLrR[:, 128 * r: 128 * r + 128]
            def klr_i(r): return KLiR[:, 128 * r: 128 * r + 128]

            _evac_flip = [0]

            def evac(dst, src):
                # PSUM -> SBUF, alternating engines for balance
                if _evac_flip[0] % 2 == 0:
                    nc.vector.tensor_copy(dst, src)
                else:
                    nc.scalar.copy(dst, src)
                _evac_flip[0] += 1

            def evac_r(dst, src):
                # PSUM -> SBUF with f32r rounding on write
                d = dst.bitcast(F32R) if F32R_ADVANCE else dst
                if _evac_flip[0] % 2 == 0:
                    nc.vector.tensor_copy(d, src)
                else:
                    nc.scalar.copy(d, src)
                _evac_flip[0] += 1

            def scratch_neg(src):
                t = pp.tile([DIM, DIM], F32, tag="negscr", name="negscr", bufs=4)
                nc.vector.tensor_scalar_mul(t[:], src, -1.0)
                return t

            # ============ stage 1: h_mat -> M = H0*DT (Walsh build) ========
            with nc.named_scope("h0"):
                psT = ppX.tile([DIM, 512], F32, tag="px", name="psT")
                nc.tensor.matmul(psT[:, 0:1], sb['chT'][:], sb['h49'][:],
                                 start=True, stop=True)
                chat = ptile("chat", [DIM, 1])
                nc.vector.tensor_copy(chat[:], psT[:, 0:1])
                nc.tensor.matmul(psT[:, 4:5], sb['dvT'][:], sb['h49'][:],
                                 start=True, stop=True)
                dvec = ptile("dvec", [DIM, 1])
                nc.scalar.copy(dvec[:], psT[:, 4:5])
                Dc = ptile("Dc", [DIM, DIM])
                nc.vector.tensor_scalar_mul(Dc[:], sb['had'][:], chat[:])
                nc.tensor.matmul(psT[:, 128:256], sb['had'][:], Dc[:],
                                 start=True, stop=True)
                CBsb = ptile("CBsb", [DIM, DIM])
                nc.vector.tensor_copy(CBsb[:], psT[:, 128:256])
                M1 = ptile("M1", [DIM, DIM])
                nc.vector.tensor_mul(M1[:], CBsb[:], sb['bvdt'][:])
                Mt = ptile("Mt", [DIM, DIM])
                nc.vector.scalar_tensor_tensor(
                    Mt[:], sb['ident'][:], dvec[:], M1[:], ALU.mult, ALU.add)

            # ============ stage 2: Taylor P = cos(M) - i sin(M) ============
            # |W| ~ 3e-4: W^3 terms are ~1e-13, below fp32 -> 2-term series
            with nc.named_scope("taylor"):
                ps = ppA.tile([DIM, 512], F32, tag="mm", name="psW")
                nc.tensor.matmul(ps[:, 0:128], Mt[:], Mt[:], start=True, stop=True)
                W = ptile("W", [DIM, DIM])
                evac(W[:], ps[:, 0:128])
                ps = ppA.tile([DIM, 512], F32, tag="mm", name="psW2")
                nc.tensor.matmul(ps[:, 0:128], W[:], W[:], start=True, stop=True)
                W2 = ptile("W2", [DIM, DIM])
                evac(W2[:], ps[:, 0:128])
                # K1 real = I - W/2 + W2/24
                t1 = ptile("tay1", [DIM, DIM])
                nc.vector.scalar_tensor_tensor(
                    t1[:], W[:], -0.5, sb['ident'][:], ALU.mult, ALU.add)
                nc.vector.scalar_tensor_tensor(
                    KLr[:, 128:256], W2[:], 1.0 / 24, t1[:], ALU.mult, ALU.add)
                # negE = W/6 - I - W2/120 ; K1 imag = M @ negE
                t3 = ptile("tay3", [DIM, DIM])
                nc.vector.scalar_tensor_tensor(
                    t3[:], W[:], 1.0 / 6, sb['ident'][:], ALU.mult, ALU.subtract)
                negE = ptile("negE", [DIM, DIM])
                nc.vector.scalar_tensor_tensor(
                    negE[:], W2[:], -1.0 / 120, t3[:], ALU.mult, ALU.add)
                ps = ppA.tile([DIM, 512], F32, tag="mm", name="psKi")
                nc.tensor.matmul(ps[:, 0:128], Mt[:], negE[:], start=True, stop=True)
                evac(KLi[:, 128:256], ps[:, 0:128])
                nc.vector.tensor_copy(KLr[:, 0:128], sb['ident'][:])
                nc.vector.memset(KLi[:, 0:128], 0.0)

            # ============ stage 3: ladder ============
            def rnd(ap):
                return ap.bitcast(F32R) if F32R_ADVANCE else ap

            def arrange_kb(p, mixed):
                """Rounded f32r arrangements of block p for the advance stage."""
                nc.vector.tensor_scalar_mul(rnd(kbn_r(p)), kb_r(p), -1.0)
                nc.vector.tensor_scalar_mul(rnd(kbn_i(p)), kb_i(p), -1.0)
                if mixed:
                    nc.scalar.copy(rnd(kbq_r(p)), kb_r(p))
                    ri = kbm_ri(p)
                    nc.vector.tensor_copy(rnd(ri[:, 0:128]), kb_r(p))
                    nc.vector.tensor_scalar_mul(rnd(ri[:, 128:256]),
                                                kb_i(p), -1.0)
                    ri = kbs_ri(p)
                    nc.scalar.copy(rnd(ri[:, 0:128]), kb_i(p))
                    nc.scalar.copy(rnd(ri[:, 128:256]), kb_r(p))

            with nc.named_scope("ladder"):
                for n in [1, 2, 4, 8]:
                    lo, hi = n + 1, min(2 * n, 16)
                    w = (hi - lo + 1) * 128
                    nki = scratch_neg(kl_i(n))
                    for base in range(0, w, 512):
                        cw = min(512, w - base)
                        boff = 128 + base
                        psR = ppA.tile([DIM, 512], F32, tag="mm", name="psLr")
                        psI = ppA.tile([DIM, 512], F32, tag="mm", name="psLi")
                        nc.tensor.matmul(psR[:, 0:cw], kl_r(n),
                                         KLr[:, boff:boff + cw],
                                         start=True, stop=False)
                        nc.tensor.matmul(psR[:, 0:cw], nki[:],
                                         KLi[:, boff:boff + cw],
                                         start=False, stop=True)
                        nc.tensor.matmul(psI[:, 0:cw], kl_i(n),
                                         KLr[:, boff:boff + cw],
                                         start=True, stop=False)
                        nc.tensor.matmul(psI[:, 0:cw], kl_r(n),
                                         KLi[:, boff:boff + cw],
                                         start=False, stop=True)
                        doff = 128 * lo + base
                        dw = min(cw, 2048 - doff) if doff < 2048 else 0
                        if dw > 0:
                            evac(KLr[:, doff:doff + dw], psR[:, 0:dw])
                            evac(KLi[:, doff:doff + dw], psI[:, 0:dw])
                        if dw < cw:      # tail target is K16 -> KB block 0
                            evac(kb_r(16), psR[:, dw:cw])
                            evac(kb_i(16), psI[:, dw:cw])
                arrange_kb(16, mixed=True)
                nknext = scratch_neg(kb_i(16))
                for p in [32, 64, 128, 256, 512, 1024, 2048]:
                    h = p // 2
                    nkh = nknext
                    ps = ppA.tile([DIM, 512], F32, tag="mm", name="psSq")
                    nc.tensor.matmul(ps[:, 0:128], kb_r(h), kb_r(h),
                                     start=True, stop=False)
                    nc.tensor.matmul(ps[:, 0:128], nkh[:], kb_i(h),
                                     start=False, stop=True)
                    nc.tensor.matmul(ps[:, 128:256], kb_i(h), kb_r(h),
                                     start=True, stop=False)
                    nc.tensor.matmul(ps[:, 128:256], kb_r(h), kb_i(h),
                                     start=False, stop=True)
                    nc.vector.tensor_copy(kb_r(p), ps[:, 0:128])
                    nc.scalar.copy(kb_i(p), ps[:, 128:256])
                    if p != 2048:
                        nknext = pp.tile([DIM, DIM], F32, tag="negscr",
                                         name="negscr", bufs=4)
                        nc.vector.tensor_scalar_mul(nknext[:],
                                                    ps[:, 128:256], -1.0)
                    if p in (64, 128, 256):
                        arrange_kb(p, mixed=True)
                if F32R_OPR:
                    nc.vector.tensor_copy(KLrR[:].bitcast(F32R), KLr[:])
                    nc.scalar.copy(KLiR[:].bitcast(F32R), KLi[:])
                else:
                    nc.vector.tensor_copy(KLrR[:], KLr[:])
                    nc.scalar.copy(KLiR[:], KLi[:])

            # ============ stage 4: per-core Q & rho start ============
            with nc.named_scope("start"):
                Ssel = ptile("Ssel", [DIM, 6 * 128])    # (Sr|Si) x 3
                nSsel = ptile("nSsel", [DIM, 3 * 128])  # -Si x 3
                for b, p in enumerate([512, 1024, 2048]):
                    tb = ptile("selt%d" % b, [DIM, DIM])
                    nc.vector.tensor_scalar_mul(
                        tb[:], sb['ident'][:], sb['bits'][:, 3 + b:4 + b])
                    nc.vector.scalar_tensor_tensor(
                        Ssel[:, 256 * b:256 * b + 128], kb_r(p),
                        sb['bits'][:, b:b + 1], tb[:], ALU.mult, ALU.add)
                    nc.vector.tensor_scalar_mul(
                        Ssel[:, 256 * b + 128:256 * b + 256], kb_i(p),
                        sb['bits'][:, b:b + 1])
                    nc.vector.tensor_scalar_mul(
                        nSsel[:, 128 * b:128 * b + 128], kb_i(p),
                        sb['bits'][:, 6 + b:7 + b])

                def cmul_sym(dstR, dstI, Ar, Ai, nAi, Br, Bi,
                             neg_out_i=False, nAr=None, round_out=False):
                    if round_out and F32R_ADVANCE:
                        dstR = dstR.bitcast(F32R)
                        dstI = dstI.bitcast(F32R)
                    ps = ppA.tile([DIM, 512], F32, tag="mm", name="psCm")
                    nc.tensor.matmul(ps[:, 0:128], Ar, Br, start=True, stop=False)
                    nc.tensor.matmul(ps[:, 0:128], nAi, Bi, start=False, stop=True)
                    if neg_out_i:
                        nc.tensor.matmul(ps[:, 128:256], nAi, Br,
                                         start=True, stop=False)
                        nc.tensor.matmul(ps[:, 128:256], nAr, Bi,
                                         start=False, stop=True)
                    else:
                        nc.tensor.matmul(ps[:, 128:256], Ai, Br,
                                         start=True, stop=False)
                        nc.tensor.matmul(ps[:, 128:256], Ar, Bi,
                                         start=False, stop=True)
                    evac(dstR, ps[:, 0:128])
                    evac(dstI, ps[:, 128:256])

                C12r = ptile("C12r", [DIM, DIM])
                C12i = ptile("C12i", [DIM, DIM])
                cmul_sym(C12r[:], C12i[:],
                         Ssel[:, 0:128], Ssel[:, 128:256], nSsel[:, 0:128],
                         Ssel[:, 256:384], Ssel[:, 384:512])
                nC12i = scratch_neg(C12i[:])
                Qr = ptile("Qr", [DIM, DIM])
                Qi = ptile("Qi", [DIM, DIM])
                cmul_sym(Qr[:], Qi[:], C12r[:], C12i[:], nC12i[:],
                         Ssel[:, 512:640], Ssel[:, 640:768])
                nQr = scratch_neg(Qr[:])
                nQi = scratch_neg(Qi[:])
                # Y = rho0 @ conj(Q): Yr = rho0@Qr ; Yi = -rho0@Qi
                psY = ppA.tile([DIM, 512], F32, tag="mm", name="psY")
                nc.tensor.matmul(psY[:, 0:128], sb['rho0'][:], Qr[:],
                                 start=True, stop=True)
                nc.tensor.matmul(psY[:, 128:256], sb['rho0'][:], Qi[:],
                                 start=True, stop=True)
                Yr = ptile("Yr", [DIM, DIM])
                Yi = ptile("Yi", [DIM, DIM])
                nc.vector.tensor_copy(Yr[:], psY[:, 0:128])
                nc.vector.tensor_scalar_mul(Yi[:], psY[:, 128:256], -1.0)
                # rho = Q @ Y -> working chain j=0 as (Rr, -Ri)
                cmul_sym(STK[:, 0:128], STK[:, 128:256],
                         Qr[:], Qi[:], nQi[:], Yr[:], Yi[:],
                         neg_out_i=True, nAr=nQr[:], round_out=True)

            # ============ stage 5: spawns + q-loop ============
            # STK snapshot-major: slot s = 8j + q at cols 256*s (Rr | -Ri)
            def advance_round(dst_slots, src_slots, p):
                width = len(src_slots)
                w = width * 128
                psZ = None
                for idx, s_src in enumerate(src_slots):
                    lR = STK[:, s_src * 256: s_src * 256 + 128]
                    lN = STK[:, s_src * 256 + 128: s_src * 256 + 256]
                    if idx % 2 == 0:
                        psZ = ppA.tile([DIM, 512], F32, tag="mm", name="psZ")
                    sl = psZ[:, (idx % 2) * 256: (idx % 2) * 256 + 256]
                    nc.tensor.matmul(sl, adv(lR), adv(kbm_ri(p)),
                                     start=True, stop=False)
                    nc.tensor.matmul(sl, adv(lN), adv(kbs_ri(p)),
                                     start=False, stop=True)
                    evac_r(Zr[:, idx * 128:(idx + 1) * 128], sl[:, 0:128])
                    evac_r(Zi[:, idx * 128:(idx + 1) * 128], sl[:, 128:256])
                for h0_ in range(0, w, 512):
                    hw_ = min(512, w - h0_)
                    psR = ppB.tile([DIM, 512], F32, tag="mm2", name="psR")
                    psN = ppB.tile([DIM, 512], F32, tag="mm2", name="psN")
                    nc.tensor.matmul(psR[:, 0:hw_], adv(kbq_r(p)),
                                     adv(Zr[:, h0_:h0_ + hw_]),
                                     start=True, stop=False)
                    nc.tensor.matmul(psR[:, 0:hw_], adv(kbn_i(p)),
                                     adv(Zi[:, h0_:h0_ + hw_]),
                                     start=False, stop=True)
                    nc.tensor.matmul(psN[:, 0:hw_], adv(kbn_r(p)),
                                     adv(Zi[:, h0_:h0_ + hw_]),
                                     start=True, stop=False)
                    nc.tensor.matmul(psN[:, 0:hw_], adv(kbn_i(p)),
                                     adv(Zr[:, h0_:h0_ + hw_]),
                                     start=False, stop=True)
                    for k_ in range(hw_ // 128):
                        s_dst = dst_slots[h0_ // 128 + k_]
                        evac_r(STK[:, s_dst * 256: s_dst * 256 + 128],
                               psR[:, 128 * k_:128 * k_ + 128])
                        evac_r(STK[:, s_dst * 256 + 128: s_dst * 256 + 256],
                               psN[:, 128 * k_:128 * k_ + 128])

            with nc.named_scope("evolve"):
                # chain j covers t0 + 64*j; slot s = 4*j + q (q < 4)
                advance_round([16], [0], 256)                  # j4 = j0+256
                advance_round([8, 24], [0, 16], 128)           # j2, j6
                advance_round([4, 12, 20, 28], [0, 8, 16, 24], 64)   # j1,3,5,7
                for q in range(1, 4):
                    advance_round([4 * j + q for j in range(8)],
                                  [4 * j + q - 1 for j in range(8)],
                                  16)

            # ============ stage 6: Op_r stack ============
            with nc.named_scope("opr"):
                def r_out(ap):
                    return ap.bitcast(F32R) if F32R_OPR else ap
                mlv = MLri[:].rearrange("p (r b) -> p r b", b=128)
                swv = MLsw[:].rearrange("p (r b) -> p r b", b=128)
                for b4 in range(4):
                    lo = 512 * b4
                    psm = ppA.tile([DIM, 512], F32, tag="mm", name="psm")
                    psn = ppA.tile([DIM, 512], F32, tag="mm", name="psn")
                    nc.tensor.matmul(psm[:, 0:512], opr(sb['opt'][:]),
                                     opr(KLrR[:, lo:lo + 512]),
                                     start=True, stop=True)
                    nc.tensor.matmul(psn[:, 0:512], opr(sb['opt'][:]),
                                     opr(KLiR[:, lo:lo + 512]),
                                     start=True, stop=True)
                    pv = psm[:, 0:512].rearrange("p (r b) -> p r b", b=128)
                    nv = psn[:, 0:512].rearrange("p (r b) -> p r b", b=128)
                    r0 = 8 * b4
                    nc.vector.tensor_copy(r_out(mlv[:, r0:r0 + 8:2, :]), pv)
                    nc.scalar.copy(r_out(mlv[:, r0 + 1:r0 + 8:2, :]), nv)
                    nc.vector.tensor_copy(r_out(swv[:, r0:r0 + 8:2, :]), nv)
                    nc.scalar.mul(r_out(swv[:, r0 + 1:r0 + 8:2, :]), pv, -1.0)
                for r in range(16):
                    c0 = 128 * r
                    pso = ppA.tile([DIM, 512], F32, tag="mm", name="pso")
                    nc.tensor.matmul(pso[:, 0:256], opr(klr_r(r)),
                                     opr(MLri[:, 256 * r:256 * r + 256]),
                                     start=True, stop=False)
                    nc.tensor.matmul(pso[:, 0:256], opr(klr_i(r)),
                                     opr(MLsw[:, 256 * r:256 * r + 256]),
                                     start=False, stop=True)
                    evac_r(OST[:, c0:c0 + 128], pso[:, 0:128])
                    evac_r(OST[:, 2048 + c0:2048 + c0 + 128], pso[:, 128:256])

            # ============ stage 7: extraction ============
            with nc.named_scope("extract"):
                psPx = ppX.tile([64, 32], F32, tag="px", name="psPx")
                stk4 = STK[:].rearrange("p (s x b) -> p s x b", s=32, x=2)
                ost4 = OST[:].rearrange("p (x r b) -> p x r b", x=2, r=16)
                for b in range(DIM):
                    nc.tensor.matmul(psPx[:], adv(stk4[:, :, :, b]),
                                     adv(ost4[:, :, :, b]),
                                     start=(b == 0), stop=(b == DIM - 1))
                Pxsb = ptile("Pxsb", [64, 32])
                nc.vector.tensor_copy(Pxsb[:], psPx[:])
                pst12 = ppB.tile([32, 64], F32, tag="mm2", name="pst12")
                nc.tensor.matmul(pst12[:, 0:32], sb['selp'][:, 0:32], Pxsb[:],
                                 start=True, stop=True)
                nc.tensor.matmul(pst12[:, 32:64], sb['selp'][:, 32:64], Pxsb[:],
                                 start=True, stop=True)
                T12 = ptile("T12", [32, 64])
                nc.vector.tensor_copy(T12[:], pst12[:])
                fra = ptile("fra", [32, 16])
                fia = ptile("fia", [32, 16])
                fre = ptile("fre", [32, 16])
                fim = ptile("fim", [32, 16])
                nc.vector.tensor_sub(fra[:], T12[:, 0:16], T12[:, 48:64])
                nc.vector.tensor_add(fia[:], T12[:, 16:32], T12[:, 32:48])
                if F32R_FFT:
                    nc.vector.tensor_mul(fre[:].bitcast(F32R), fra[:],
                                         sb['apod'][:])
                    nc.vector.tensor_mul(fim[:].bitcast(F32R), fia[:],
                                         sb['apod'][:])
                else:
                    nc.vector.tensor_mul(fre[:], fra[:], sb['apod'][:])
                    nc.vector.tensor_mul(fim[:], fia[:], sb['apod'][:])
                nc.sync.dma_start(out_fidr[:], fre[:])
                nc.sync.dma_start(out_fidi[:], fim[:])

            # ============ stage 8: partial spectrum (3-stage DFT) ============
            with nc.named_scope("fft"):
                psYa = ppB.tile([16, 512], F32, tag="mm2", name="psYa")
                nc.tensor.matmul(psYa[:], fft(fre[:]), fft(sb['far'][:]),
                                 start=True, stop=False)
                nc.tensor.matmul(psYa[:], fft(fim[:]), fft(sb['fain'][:]),
                                 start=False, stop=True)
                Yfr = ptile("Yfr", [16, 512])
                nc.vector.tensor_copy(Yfr[:], psYa[:])
                psYb = ppB.tile([16, 512], F32, tag="mm2", name="psYb")
                nc.tensor.matmul(psYb[:], fft(fre[:]), fft(sb['fai'][:]),
                                 start=True, stop=False)
                nc.tensor.matmul(psYb[:], fft(fim[:]), fft(sb['far'][:]),
                                 start=False, stop=True)
                Yfi = ptile("Yfi", [16, 512])
                nc.vector.tensor_copy(Yfi[:], psYb[:])
                # Z = Y o TW (elementwise complex, SBUF only)
                za = ptile("za", [16, 512])
                zb = ptile("zb", [16, 512])
                Zfr = ptile("Zfr", [16, 512])
                Zfi = ptile("Zfi", [16, 512])
                nc.vector.tensor_mul(za[:], Yfr[:], sb['twr'][:])
                nc.vector.tensor_mul(zb[:], Yfi[:], sb['twi'][:])
                nc.vector.tensor_sub(
                    Zfr[:].bitcast(F32R) if F32R_FFT else Zfr[:],
                    za[:], zb[:])
                nc.vector.tensor_mul(za[:], Yfr[:], sb['twi'][:])
                nc.vector.tensor_mul(zb[:], Yfi[:], sb['twr'][:])
                nc.vector.tensor_add(
                    Zfi[:].bitcast(F32R) if F32R_FFT else Zfi[:],
                    za[:], zb[:])
                psSa = ppB.tile([16, 512], F32, tag="mm2", name="psSa")
                nc.tensor.matmul(psSa[:], fft(sb['fcr'][:]), fft(Zfr[:]),
                                 start=True, stop=False)
                nc.tensor.matmul(psSa[:], fft(sb['fcin'][:]), fft(Zfi[:]),
                                 start=False, stop=True)
                spR = ptile("spR", [16, 512])
                nc.vector.tensor_copy(spR[:], psSa[:])
                psSb = ppB.tile([16, 512], F32, tag="mm2", name="psSb")
                nc.tensor.matmul(psSb[:], fft(sb['fci'][:]), fft(Zfr[:]),
                                 start=True, stop=False)
                nc.tensor.matmul(psSb[:], fft(sb['fcr'][:]), fft(Zfi[:]),
                                 start=False, stop=True)
                spI = ptile("spI", [16, 512])
                nc.vector.tensor_copy(spI[:], psSb[:])
                nc.sync.dma_start(out_spr[:], spR[:])
                nc.sync.dma_start(out_spi[:], spI[:])

    nc.compile()
    return nc


# ----------------------------------------------------------------------
# entry point
# ----------------------------------------------------------------------
def kernel(h_mat, n_samp):
    from concourse.bass_utils import run_bass_kernel_spmd

    n_samp = int(n_samp)
    assert n_samp == NS, f"kernel compiled for n_samp={NS}, got {n_samp}"
    h_mat = np.asarray(h_mat, dtype=np.float32)
    assert h_mat.shape == (7, 7)

    nc = _build_nc()
    consts = _consts()
    in_maps = []
    for c in range(N_CORES):
        m = dict(consts)
        m.update(_core_inputs(h_mat, c))
        in_maps.append(m)
    res = run_bass_kernel_spmd(nc, in_maps, core_ids=list(range(N_CORES)))

    # gather / unshard
    fid = np.zeros(NS, dtype=np.complex64)
    spec = np.zeros(2 * NS, dtype=np.complex64)
    for c in range(N_CORES):
        r = res.results[c]
        fid[512 * c: 512 * (c + 1)] = (
            r["fidr"].reshape(-1) + 1j * r["fidi"].reshape(-1))
        spec += r["spr"].reshape(-1) + 1j * r["spi"].reshape(-1)

    aq = NS / SW
    time_series = np.linspace(0.0, aq, NS, dtype=np.float32)
    freq_series = np.linspace(-SW / 2, SW / 2, 2 * NS, dtype=np.float32)
    return fid, time_series, spec, freq_series
